# revision 1
# baseline (speedup 1.0000x reference)
"""Trainium2 Bass kernel for nn_BiochemicalDiffusion.

Computes  out = F - B*x - r * rowsum(x * (A @ x))  for A:[10000,10000] f32,
x:[10000,64] f32, across 8 NeuronCores.

Sharding (all done host-side in this file):
  - A is sharded row-wise: core c gets rows [c*1250, (c+1)*1250).
  - The shard is passed pre-transposed (A_shard^T, [10000, 1250]) so the PE
    can contract over k directly: Ax_shard = A_shard^T.T @ x.
  - x is passed in full to every core (it is tiny), pre-tiled into the
    [128, 79*64] SBUF layout the matmul consumes.
  - Each core computes its [1250, 64] slice of the output; the host
    concatenates them.

Hardware note: PSUM accumulation groups must not share a PSUM bank — two
interleaved accumulation groups in one bank corrupt each other.  Both
layouts below keep one live accumulation group per bank.

Everything is hardcoded to the problem shapes; kernel.py is self-contained.
"""

import numpy as np

N = 10000
DIM = 64
NCORES = 8
MSHARD = N // NCORES  # 1250 rows of A / out per core
MT = 125              # m-tile (PSUM partition) size
NMT = MSHARD // MT    # 10 m-tiles per core
KT = 128              # k-tile (contraction) size
NKT = 79              # k-tiles covering the 10000 rows (last is 16+zeros)
KPAD = NKT * KT       # 10112 (rows 10000+ are zeros; they contribute 0)

F_CONST = 1.0
B_CONST = 0.1
R_CONST = 0.01

# m-chunks for the x-stationary layout (moving free dim >= 256 keeps fp32r
# at full rate; each chunk's accumulator owns one PSUM bank; widths must be
# EVEN -- fp32r matmul ISA restriction on innermost free counts)
MCH = [(0, 418), (418, 834), (834, 1250)]

# k-tile DMA groups: up to 4 k-tiles per transfer (~1.3-2.6 MB).  A^T is
# pre-tiled on the HOST into exactly this slab layout (group-major,
# partition-major inside a group) so each group is ONE flat contiguous
# 2D DMA -- large per-partition bursts, minimal descriptor work.  The
# first groups are deliberately small so the first matmul starts early
# (pipeline ramp), the steady state uses full quads.
KQ = 4
KGROUPS = ([(0, 1), (1, 1), (2, 2)]
           + [(k0, 4) for k0 in range(4, 76, 4)]
           + [(76, 3)])
NG = len(KGROUPS)                     # 22 groups covering 79 tiles

A_LO_SCALE = 512.0  # fp8 A_lo is stored pre-scaled into [-1, 1]

DEFAULT_LAYOUT = "x_stat"     # only x_stat is implemented
DEFAULT_MM_DTYPE = "splitf8"  # "f32r" | "bf16" | "split" | "splitf8"

_nc_cache = {}


def _dtypes(mm_dtype):
    from concourse import mybir
    mm = {
        "f32": mybir.dt.float32,
        "f32r": mybir.dt.float32r,
        "bf16": mybir.dt.bfloat16,
        "split": mybir.dt.bfloat16,
        "splitf8": mybir.dt.bfloat16,
    }[mm_dtype]
    return mm, mybir.dt.float32


def _np_mm_dtype(mm_dtype):
    if mm_dtype in ("bf16", "split", "splitf8"):
        import ml_dtypes
        return np.dtype(ml_dtypes.bfloat16)
    return np.dtype(np.float32)


def _body_x_stat(ctx, tc, a_t, a_l, xt_d, xt8_d, xs_d, xst_d, id_d, out_d,
                 mmdt, mm_dtype):
    """k-outer loop; x k-tiles are the stationary operand, A^T slabs stream
    as the moving operand (large free dim -> full-rate fp32r / bf16).
    Produces Ax^T in PSUM (3 chunk accumulators, one bank each); epilogue
    transposes x^T*Ax^T back via the PE.

    DMA streams in KQ-k-tile groups (~1.3-2.6 MB per transfer) to amortize
    per-DMA overhead; the stationary x is preloaded in per-group chunks on
    the gpsimd queue so the first matmul does not wait for the whole x.

    split: A and x decomposed as hi+lo bf16 pairs; A@x ~= A_hi@x_hi +
    A_lo@x_hi + A_hi@x_lo.  a_t holds [A_hi^T | A_lo^T] side by side; xt
    holds [x_hi | x_lo] per k-tile so the two x terms ride in ONE 128-wide
    stationary: pass A computes both x_hi@A_hi (psum rows 0:64) and
    x_lo@A_hi (rows 64:128) in a single moving sweep of the A_hi slab
    half; pass B computes x_hi@A_lo.

    splitf8: like split but A_lo is a SEPARATE fp8e4m3 tensor pre-scaled
    by A_LO_SCALE, and pass B runs all-fp8 (x in fp8) -- 3 bytes/element
    of A traffic instead of 4; epilogue rescales the pass-B accumulator."""
    import concourse.bass  # noqa: F401
    from concourse import mybir

    nc = tc.nc
    f32 = mybir.dt.float32
    fp8 = mybir.dt.float8e4
    split = mm_dtype in ("split", "splitf8")
    f8 = mm_dtype == "splitf8"

    consts = ctx.enter_context(tc.tile_pool(name="consts", bufs=1))
    slabs = ctx.enter_context(tc.tile_pool(name="slabs", bufs=6))
    psums = ctx.enter_context(tc.tile_pool(name="psums", bufs=1, space="PSUM"))
    ptp = ctx.enter_context(tc.tile_pool(name="ptp", bufs=2, space="PSUM"))
    epil = ctx.enter_context(tc.tile_pool(name="epil", bufs=2))

    # elements per k-row in the a_t tensor.  For splitf8 the hi (bf16) and
    # lo (fp8) halves are byte-packed into one bf16-typed stream:
    # per k-tile per partition = 1250 bf16 hi elems then 1250 fp8 lo bytes
    # (= 625 bf16-elem slots); pass B reads the lo region via bitcast.
    awid = 2 * MSHARD if (split and not f8) else MSHARD
    if f8:
        awid = MSHARD + MSHARD // 2  # 1875 bf16 elems per k-tile
    xwid = 2 * DIM if split else DIM  # stationary block width per k-tile

    xt = consts.tile([KT, NKT * xwid], mmdt)
    if f8:
        xt8 = consts.tile([KT, NKT * DIM], fp8)
    bcol = consts.tile([MT, 1], f32)
    nc.vector.memset(bcol, -B_CONST)

    accs = [psums.tile([xwid, c1 - c0], f32, name=f"acc{i}", tag=f"acc{i}")
            for i, (c0, c1) in enumerate(MCH)]
    if split:
        accs_lo = [psums.tile([DIM, c1 - c0], f32, name=f"accl{i}",
                              tag=f"accl{i}")
                   for i, (c0, c1) in enumerate(MCH)]

    for gi, (k0, g) in enumerate(KGROUPS):
        # stationary chunk for this group's k-tiles (gpsimd queue, overlaps
        # with the slab stream on the sync queue)
        nc.gpsimd.dma_start(
            out=xt[:, k0 * xwid:(k0 + g) * xwid],
            in_=xt_d[:, k0 * xwid:(k0 + g) * xwid],
        )
        if f8:
            nc.gpsimd.dma_start(
                out=xt8[:, k0 * DIM:(k0 + g) * DIM],
                in_=xt8_d[:, k0 * DIM:(k0 + g) * DIM],
            )
        slab = slabs.tile([KT, KQ * awid], mmdt, name=f"slab{gi}", tag="slab")
        nc.sync.dma_start(out=slab[:, :g * awid],
                          in_=a_t[gi * KT:(gi + 1) * KT, :g * awid])

        for sub in range(g):
            kt = k0 + sub
            xoff = kt * xwid
            base = sub * MSHARD if f8 else sub * awid
            for i, (c0, c1) in enumerate(MCH):
                # pass A: [x_hi | x_lo] (or plain x) against the A_hi half
                nc.tensor.matmul(
                    accs[i],
                    lhsT=xt[:, xoff:xoff + xwid],
                    rhs=slab[:, base + c0:base + c1],
                    start=(kt == 0),
                    stop=(kt == NKT - 1),
                )
            if split:
                for i, (c0, c1) in enumerate(MCH):
                    # pass B: x_hi (bf16) or x (fp8) against the A_lo half
                    if f8:
                        off = g * MSHARD + (sub * MSHARD + c0) // 2
                        rhs = slab[:, off:off + (c1 - c0) // 2].bitcast(fp8)
                        lo_lhs = xt8[:, kt * DIM:(kt + 1) * DIM]
                    else:
                        rhs = slab[:, base + MSHARD + c0:base + MSHARD + c1]
                        lo_lhs = xt[:, xoff:xoff + DIM]
                    nc.tensor.matmul(
                        accs_lo[i],
                        lhsT=lo_lhs,
                        rhs=rhs,
                        start=(kt == 0),
                        stop=(kt == NKT - 1),
                    )

    # epilogue-only constants: issued after the slab stream in program
    # order so they don't delay the first matmuls; they transfer during
    # the main loop and are ready long before the epilogue needs them.
    xs = consts.tile([MT, NMT * DIM], f32)
    nc.gpsimd.dma_start(out=xs, in_=xs_d)
    xst = consts.tile([DIM, MSHARD], f32)
    nc.gpsimd.dma_start(out=xst, in_=xst_d)
    ident = consts.tile([DIM, DIM], f32)
    nc.gpsimd.dma_start(out=ident, in_=id_d)

    # P = x^T * Ax^T  (elementwise), [64, 1250] in SBUF
    p_full = epil.tile([DIM, MSHARD], f32, bufs=1)
    for i, (c0, c1) in enumerate(MCH):
        w = c1 - c0
        if split:
            # only one PSUM operand allowed per DVE op -> chain via SBUF
            tsum = epil.tile([DIM, w], f32, name=f"tsum{i}", tag="tsum")
            nc.vector.tensor_copy(tsum, accs[i][0:DIM, :])
            nc.vector.tensor_add(tsum, tsum, accs[i][DIM:2 * DIM, :])
            if f8:
                tlo = epil.tile([DIM, w], f32, name=f"tlo{i}", tag="tlo")
                nc.vector.tensor_scalar(
                    out=tlo, in0=accs_lo[i], scalar1=1.0 / A_LO_SCALE,
                    scalar2=None, op0=mybir.AluOpType.mult)
                nc.vector.tensor_add(tsum, tsum, tlo)
            else:
                nc.vector.tensor_add(tsum, tsum, accs_lo[i])
            nc.vector.tensor_mul(p_full[:, c0:c1], xst[:, c0:c1], tsum)
        else:
            nc.vector.tensor_mul(p_full[:, c0:c1], xst[:, c0:c1], accs[i])

    for mt in range(NMT):
        pt = ptp.tile([MT, DIM], f32, name=f"pt{mt}", tag="pt")
        nc.tensor.transpose(
            out=pt, in_=p_full[:, mt * MT:(mt + 1) * MT], identity=ident,
        )
        s = epil.tile([MT, 1], f32, name=f"s{mt}", tag="s")
        nc.vector.tensor_reduce(
            out=s, in_=pt, axis=mybir.AxisListType.X, op=mybir.AluOpType.add,
        )
        t_col = epil.tile([MT, 1], f32, name=f"t{mt}", tag="t")
        # t = s * (-r) + F
        nc.vector.tensor_scalar(
            out=t_col, in0=s, scalar1=-R_CONST, scalar2=F_CONST,
            op0=mybir.AluOpType.mult, op1=mybir.AluOpType.add,
        )
        o = epil.tile([MT, DIM], f32, name=f"o{mt}", tag="o")
        nc.vector.tensor_scalar(
            out=o, in0=xs[:, mt * DIM:(mt + 1) * DIM], scalar1=bcol,
            scalar2=t_col, op0=mybir.AluOpType.mult, op1=mybir.AluOpType.add,
        )
        nc.sync.dma_start(out=out_d[mt * MT:(mt + 1) * MT, :], in_=o)


def build(layout=None, mm_dtype=None):
    layout = layout or DEFAULT_LAYOUT
    mm_dtype = mm_dtype or DEFAULT_MM_DTYPE
    key = (layout, mm_dtype)
    if key in _nc_cache:
        return _nc_cache[key]

    from contextlib import ExitStack
    import concourse.tile as tile
    from concourse import bacc

    mmdt, f32 = _dtypes(mm_dtype)

    nc = bacc.Bacc(
        "TRN2",
        target_bir_lowering=False,
        debug=False,
        enable_asserts=False,
        num_devices=NCORES,
        name=f"biochem_{layout}_{mm_dtype}",
    )
    from concourse import mybir

    split = mm_dtype in ("split", "splitf8")
    f8 = mm_dtype == "splitf8"
    awid = 2 * MSHARD if (split and not f8) else MSHARD
    if f8:
        awid = MSHARD + MSHARD // 2  # byte-packed hi(bf16)+lo(fp8)
    xwid = 2 * DIM if split else DIM
    # a_t is pre-tiled host-side into slab layout: row gi*128+p holds the
    # p-th partition of DMA group gi ([KQ consecutive k-rows] worth of data)
    a_t = nc.dram_tensor(
        "a_t", [NG * KT, KQ * awid], mmdt, kind="ExternalInput").ap()
    a_l = xt8_d = None
    if f8:
        xt8_d = nc.dram_tensor(
            "xt8", [KT, NKT * DIM], mybir.dt.float8e4,
            kind="ExternalInput").ap()
    xt_d = nc.dram_tensor("xt", [KT, NKT * xwid], mmdt, kind="ExternalInput").ap()
    xs_d = nc.dram_tensor("xs", [MT, NMT * DIM], f32, kind="ExternalInput").ap()
    if layout == "x_stat":
        xst_d = nc.dram_tensor("xst", [DIM, MSHARD], f32, kind="ExternalInput").ap()
        id_d = nc.dram_tensor("ident", [DIM, DIM], f32, kind="ExternalInput").ap()
    out_d = nc.dram_tensor("out", [MSHARD, DIM], f32, kind="ExternalOutput").ap()

    with tile.TileContext(nc) as tc:
        with ExitStack() as ctx:
            if layout == "x_stat":
                _body_x_stat(ctx, tc, a_t, a_l, xt_d, xt8_d, xs_d, xst_d,
                             id_d, out_d, mmdt, mm_dtype)
            else:
                raise ValueError(layout)
    nc.compile()
    _nc_cache[key] = nc
    return nc


def prepare_in_maps(x, A, layout=None, mm_dtype=None):
    layout = layout or DEFAULT_LAYOUT
    mm_dtype = mm_dtype or DEFAULT_MM_DTYPE
    np_mm = _np_mm_dtype(mm_dtype)

    x = np.asarray(x, np.float32)
    A = np.asarray(A, np.float32)
    split = mm_dtype in ("split", "splitf8")
    f8 = mm_dtype == "splitf8"
    if f8:
        import ml_dtypes
        np_fp8 = np.dtype(ml_dtypes.float8_e4m3)

    def tile_k(arr):
        """[KPAD, W] -> [KT, NKT*W] SBUF layout, padded rows are zero."""
        w = arr.shape[1]
        xp = np.zeros((KPAD, w), arr.dtype)
        xp[:N] = arr
        return np.ascontiguousarray(
            xp.reshape(NKT, KT, w).transpose(1, 0, 2).reshape(KT, NKT * w)
        )

    xt8_np = None
    if split:
        x_hi = x.astype(np_mm)
        x_lo = (x - x_hi.astype(np.float32)).astype(np_mm)
        # per k-tile stationary block is [x_hi | x_lo], 128 wide
        xt_np = tile_k(np.concatenate([x_hi, x_lo], axis=1))
        if f8:
            xt8_np = tile_k(x.astype(np_fp8))
    else:
        xt_np = tile_k(x).astype(np_mm)

    ident = np.eye(DIM, dtype=np.float32)

    def tile_slabs(at):
        """[KPAD, W] -> [NG*128, KQ*W] host pre-tiling into slab layout:
        row gi*128+p, cols sub*W:(sub+1)*W  =  at[(k0+sub)*128 + p, :]
        for group gi=(k0, g); unused columns of small groups stay zero."""
        w = at.shape[1]
        out = np.zeros((NG * KT, KQ * w), at.dtype)
        for gi, (k0, g) in enumerate(KGROUPS):
            blk = at[k0 * KT:(k0 + g) * KT, :]
            out[gi * KT:(gi + 1) * KT, :g * w] = (
                blk.reshape(g, KT, w).transpose(1, 0, 2).reshape(KT, g * w)
            )
        return out

    def pad_k(at):
        out = np.zeros((KPAD, at.shape[1]), at.dtype)
        out[:N] = at
        return out

    in_maps = []
    for c in range(NCORES):
        sh = slice(c * MSHARD, (c + 1) * MSHARD)
        at_f32 = pad_k(np.ascontiguousarray(A[sh].T))
        if f8:
            a_hi = at_f32.astype(np_mm)
            a_lo = at_f32 - a_hi.astype(np.float32)
            hi_t = tile_slabs(a_hi)                                # bf16
            lo_t = tile_slabs((a_lo * A_LO_SCALE).astype(np_fp8))  # fp8
            # byte-pack: per group row block, [g*2500 B hi][g*1250 B lo]
            awid = MSHARD + MSHARD // 2
            a_t_c = np.zeros((NG * KT, KQ * awid), np_mm)
            ob = a_t_c.view(np.uint8)
            hb = hi_t.view(np.uint8)
            lb = lo_t.view(np.uint8)
            for gi, (k0, g) in enumerate(KGROUPS):
                r = slice(gi * KT, (gi + 1) * KT)
                ob[r, :g * 2 * MSHARD] = hb[r, :g * 2 * MSHARD]
                ob[r, g * 2 * MSHARD:g * 3 * MSHARD] = lb[r, :g * MSHARD]
        elif split:
            a_hi = at_f32.astype(np_mm)
            a_lo = (at_f32 - a_hi.astype(np.float32)).astype(np_mm)
            a_t_c = tile_slabs(np.concatenate([a_hi, a_lo], axis=1))
        else:
            a_t_c = tile_slabs(at_f32.astype(np_mm))
        xs_c = np.ascontiguousarray(
            x[sh].reshape(NMT, MT, DIM).transpose(1, 0, 2).reshape(MT, NMT * DIM)
        )
        m = {"a_t": a_t_c, "xt": xt_np, "xs": xs_c}
        if f8:
            m["xt8"] = xt8_np
        if layout == "x_stat":
            m["xst"] = np.ascontiguousarray(x[sh].T)
            m["ident"] = ident
        in_maps.append(m)
    return in_maps


def run(inputs, trace=False, layout=None, mm_dtype=None, **spmd_kwargs):
    """Returns (full_output [10000, 64] f32, BassKernelResults)."""
    from concourse.bass_utils import run_bass_kernel_spmd

    nc = build(layout, mm_dtype)
    in_maps = prepare_in_maps(inputs["x"], inputs["A"], layout, mm_dtype)
    res = run_bass_kernel_spmd(
        nc, in_maps, core_ids=list(range(NCORES)), trace=trace, **spmd_kwargs
    )
    out = np.concatenate([res.results[c]["out"] for c in range(NCORES)], axis=0)
    return out, res


def kernel(t=None, x=None, A=None):
    out, _ = run({"x": x, "A": A})
    return out



# revision 5
# speedup vs baseline: 2.0670x; 2.0670x over previous
"""Trainium2 Bass kernel for nn_BiochemicalDiffusion.

Computes  out = F - B*x - r * rowsum(x * (A @ x))  for A:[10000,10000] f32,
x:[10000,64] f32, across 8 NeuronCores.

Sharding (all done host-side in this file):
  - A is sharded row-wise: core c gets rows [c*1250, (c+1)*1250).
  - The shard is passed pre-transposed (A_shard^T, [10000, 1250]) so the PE
    can contract over k directly: Ax_shard = A_shard^T.T @ x.
  - x is passed in full to every core (it is tiny), pre-tiled into the
    [128, 79*64] SBUF layout the matmul consumes.
  - Each core computes its [1250, 64] slice of the output; the host
    concatenates them.

Hardware note: PSUM accumulation groups must not share a PSUM bank — two
interleaved accumulation groups in one bank corrupt each other.  Both
layouts below keep one live accumulation group per bank.

Everything is hardcoded to the problem shapes; kernel.py is self-contained.
"""

import numpy as np

N = 10000
DIM = 64
NCORES = 8
MSHARD = N // NCORES  # 1250 rows of A / out per core
MT = 125              # m-tile (PSUM partition) size
NMT = MSHARD // MT    # 10 m-tiles per core
KT = 128              # k-tile (contraction) size
NKT = 79              # k-tiles covering the 10000 rows (last is 16+zeros)
KPAD = NKT * KT       # 10112 (rows 10000+ are zeros; they contribute 0)

F_CONST = 1.0
B_CONST = 0.1
R_CONST = 0.01

# m-chunks for the x-stationary layout (moving free dim >= 256 keeps fp32r
# at full rate; each chunk's accumulator owns one PSUM bank; widths must be
# EVEN -- fp32r matmul ISA restriction on innermost free counts)
MCH = [(0, 418), (418, 834), (834, 1250)]

# k-tile DMA groups: up to 4 k-tiles per transfer (~1.3-2.6 MB).  A^T is
# pre-tiled on the HOST into exactly this slab layout (group-major,
# partition-major inside a group) so each group is ONE flat contiguous
# 2D DMA -- large per-partition bursts, minimal descriptor work.  The
# first groups are deliberately small so the first matmul starts early
# (pipeline ramp), the steady state uses full quads.
KQ = 4
KGROUPS = ([(0, 1), (1, 1), (2, 2)]
           + [(k0, 4) for k0 in range(4, 76, 4)]
           + [(76, 3)])
NG = len(KGROUPS)                     # 22 groups covering 79 tiles

A_LO_SCALE = 512.0  # fp8 A_lo is stored pre-scaled into [-1, 1]

# f8dr variant: A entirely fp8e4 (1 B/elem of HBM traffic), matmuls in
# DoubleRow perf mode (2 k-tiles = 256 contraction rows per instruction at
# 0.5 cycles per output row).  k-tiles 0..77 ride in pairs; tile 78 (the
# 16-real-rows remainder) is a trailing single-row matmul.  Groups must be
# even-aligned and even-sized so pairs never straddle a slab boundary.
KQ_F8 = 8
KGROUPS_F8 = [(0, 2), (2, 2), (4, 4), (8, 6), (14, 8), (22, 8), (30, 8),
              (38, 8), (46, 8), (54, 8), (62, 8), (70, 8), (78, 1)]
NG_F8 = len(KGROUPS_F8)
assert sum(g for _, g in KGROUPS_F8) == NKT

DEFAULT_LAYOUT = "x_stat"    # only x_stat is implemented
DEFAULT_MM_DTYPE = "f8dr"    # "f32r" | "bf16" | "split" | "splitf8" | "f8dr"

_nc_cache = {}


def _dtypes(mm_dtype):
    from concourse import mybir
    mm = {
        "f32": mybir.dt.float32,
        "f32r": mybir.dt.float32r,
        "bf16": mybir.dt.bfloat16,
        "split": mybir.dt.bfloat16,
        "splitf8": mybir.dt.bfloat16,
        "f8dr": mybir.dt.float8e4,
    }[mm_dtype]
    return mm, mybir.dt.float32


def _np_mm_dtype(mm_dtype):
    if mm_dtype in ("bf16", "split", "splitf8"):
        import ml_dtypes
        return np.dtype(ml_dtypes.bfloat16)
    if mm_dtype == "f8dr":
        import ml_dtypes
        return np.dtype(ml_dtypes.float8_e4m3)
    return np.dtype(np.float32)


def _body_f8dr(ctx, tc, a_t, xt_d, xs_d, xst_d, id_d, out_d):
    """All-fp8 A stream with DoubleRow matmuls.

    Per slab group: one contiguous fp8 DMA ([128, g, 1250], up to 10 KB per
    partition line), then g/2 DoubleRow matmuls per m-chunk, each consuming
    2 k-tiles (256 contraction rows) of both the stationary x and the moving
    A^T slab via 3D APs [128, 2, w].  The remainder k-tile 78 (rows
    9984..10112, only 16 real) is a plain single-row fp8 matmul."""
    import concourse.bass  # noqa: F401
    from concourse import mybir

    nc = tc.nc
    f32 = mybir.dt.float32
    fp8 = mybir.dt.float8e4
    DR = mybir.MatmulPerfMode.DoubleRow

    consts = ctx.enter_context(tc.tile_pool(name="consts", bufs=1))
    slabs = ctx.enter_context(tc.tile_pool(name="slabs", bufs=4))
    psums = ctx.enter_context(tc.tile_pool(name="psums", bufs=1, space="PSUM"))
    ptp = ctx.enter_context(tc.tile_pool(name="ptp", bufs=2, space="PSUM"))
    epil = ctx.enter_context(tc.tile_pool(name="epil", bufs=2))

    xt = consts.tile([KT, NKT, DIM], fp8)
    bcol = consts.tile([MT, 1], f32)
    nc.vector.memset(bcol, -B_CONST)

    accs = [psums.tile([DIM, c1 - c0], f32, name=f"acc{i}", tag=f"acc{i}")
            for i, (c0, c1) in enumerate(MCH)]

    for gi, (k0, g) in enumerate(KGROUPS_F8):
        nc.gpsimd.dma_start(out=xt[:, k0:k0 + g, :], in_=xt_d[:, k0:k0 + g, :])
        slab = slabs.tile([KT, KQ_F8, MSHARD], fp8, name=f"slab{gi}",
                          tag="slab")
        nc.sync.dma_start(out=slab[:, :g, :],
                          in_=a_t[gi * KT:(gi + 1) * KT, :g, :])
        for sub in range(0, g - 1, 2):
            kt = k0 + sub
            for i, (c0, c1) in enumerate(MCH):
                nc.tensor.matmul(
                    accs[i],
                    lhsT=xt[:, kt:kt + 2, :],
                    rhs=slab[:, sub:sub + 2, c0:c1],
                    start=(kt == 0),
                    stop=False,
                    perf_mode=DR,
                )
        if g % 2 == 1:  # the final remainder tile
            kt = k0
            for i, (c0, c1) in enumerate(MCH):
                nc.tensor.matmul(
                    accs[i],
                    lhsT=xt[:, kt, :],
                    rhs=slab[:, 0, c0:c1],
                    start=False,
                    stop=True,
                )

    # epilogue-only constants: issued after the slab stream in program order
    # so they don't delay the first matmuls
    xs = consts.tile([MT, NMT * DIM], f32)
    nc.gpsimd.dma_start(out=xs, in_=xs_d)
    xst = consts.tile([DIM, MSHARD], f32)
    nc.gpsimd.dma_start(out=xst, in_=xst_d)
    ident = consts.tile([DIM, DIM], f32)
    nc.gpsimd.dma_start(out=ident, in_=id_d)

    # P = x^T * Ax^T  (elementwise), [64, 1250] in SBUF
    p_full = epil.tile([DIM, MSHARD], f32, bufs=1)
    for i, (c0, c1) in enumerate(MCH):
        nc.vector.tensor_mul(p_full[:, c0:c1], xst[:, c0:c1], accs[i])

    for mt in range(NMT):
        pt = ptp.tile([MT, DIM], f32, name=f"pt{mt}", tag="pt")
        nc.tensor.transpose(
            out=pt, in_=p_full[:, mt * MT:(mt + 1) * MT], identity=ident,
        )
        s = epil.tile([MT, 1], f32, name=f"s{mt}", tag="s")
        nc.vector.tensor_reduce(
            out=s, in_=pt, axis=mybir.AxisListType.X, op=mybir.AluOpType.add,
        )
        t_col = epil.tile([MT, 1], f32, name=f"t{mt}", tag="t")
        # t = s * (-r) + F
        nc.vector.tensor_scalar(
            out=t_col, in0=s, scalar1=-R_CONST, scalar2=F_CONST,
            op0=mybir.AluOpType.mult, op1=mybir.AluOpType.add,
        )
        o = epil.tile([MT, DIM], f32, name=f"o{mt}", tag="o")
        nc.vector.tensor_scalar(
            out=o, in0=xs[:, mt * DIM:(mt + 1) * DIM], scalar1=bcol,
            scalar2=t_col, op0=mybir.AluOpType.mult, op1=mybir.AluOpType.add,
        )
        nc.sync.dma_start(out=out_d[mt * MT:(mt + 1) * MT, :], in_=o)


def _body_x_stat(ctx, tc, a_t, a_l, xt_d, xt8_d, xs_d, xst_d, id_d, out_d,
                 mmdt, mm_dtype):
    """k-outer loop; x k-tiles are the stationary operand, A^T slabs stream
    as the moving operand (large free dim -> full-rate fp32r / bf16).
    Produces Ax^T in PSUM (3 chunk accumulators, one bank each); epilogue
    transposes x^T*Ax^T back via the PE.

    DMA streams in KQ-k-tile groups (~1.3-2.6 MB per transfer) to amortize
    per-DMA overhead; the stationary x is preloaded in per-group chunks on
    the gpsimd queue so the first matmul does not wait for the whole x.

    split: A and x decomposed as hi+lo bf16 pairs; A@x ~= A_hi@x_hi +
    A_lo@x_hi + A_hi@x_lo.  a_t holds [A_hi^T | A_lo^T] side by side; xt
    holds [x_hi | x_lo] per k-tile so the two x terms ride in ONE 128-wide
    stationary: pass A computes both x_hi@A_hi (psum rows 0:64) and
    x_lo@A_hi (rows 64:128) in a single moving sweep of the A_hi slab
    half; pass B computes x_hi@A_lo.

    splitf8: like split but A_lo is a SEPARATE fp8e4m3 tensor pre-scaled
    by A_LO_SCALE, and pass B runs all-fp8 (x in fp8) -- 3 bytes/element
    of A traffic instead of 4; epilogue rescales the pass-B accumulator."""
    import concourse.bass  # noqa: F401
    from concourse import mybir

    nc = tc.nc
    f32 = mybir.dt.float32
    fp8 = mybir.dt.float8e4
    split = mm_dtype in ("split", "splitf8")
    f8 = mm_dtype == "splitf8"

    consts = ctx.enter_context(tc.tile_pool(name="consts", bufs=1))
    slabs = ctx.enter_context(tc.tile_pool(name="slabs", bufs=6))
    psums = ctx.enter_context(tc.tile_pool(name="psums", bufs=1, space="PSUM"))
    ptp = ctx.enter_context(tc.tile_pool(name="ptp", bufs=2, space="PSUM"))
    epil = ctx.enter_context(tc.tile_pool(name="epil", bufs=2))

    # elements per k-row in the a_t tensor.  For splitf8 the hi (bf16) and
    # lo (fp8) halves are byte-packed into one bf16-typed stream:
    # per k-tile per partition = 1250 bf16 hi elems then 1250 fp8 lo bytes
    # (= 625 bf16-elem slots); pass B reads the lo region via bitcast.
    awid = 2 * MSHARD if (split and not f8) else MSHARD
    if f8:
        awid = MSHARD + MSHARD // 2  # 1875 bf16 elems per k-tile
    xwid = 2 * DIM if split else DIM  # stationary block width per k-tile

    xt = consts.tile([KT, NKT * xwid], mmdt)
    if f8:
        xt8 = consts.tile([KT, NKT * DIM], fp8)
    bcol = consts.tile([MT, 1], f32)
    nc.vector.memset(bcol, -B_CONST)

    accs = [psums.tile([xwid, c1 - c0], f32, name=f"acc{i}", tag=f"acc{i}")
            for i, (c0, c1) in enumerate(MCH)]
    if split:
        accs_lo = [psums.tile([DIM, c1 - c0], f32, name=f"accl{i}",
                              tag=f"accl{i}")
                   for i, (c0, c1) in enumerate(MCH)]

    for gi, (k0, g) in enumerate(KGROUPS):
        # stationary chunk for this group's k-tiles (gpsimd queue, overlaps
        # with the slab stream on the sync queue)
        nc.gpsimd.dma_start(
            out=xt[:, k0 * xwid:(k0 + g) * xwid],
            in_=xt_d[:, k0 * xwid:(k0 + g) * xwid],
        )
        if f8:
            nc.gpsimd.dma_start(
                out=xt8[:, k0 * DIM:(k0 + g) * DIM],
                in_=xt8_d[:, k0 * DIM:(k0 + g) * DIM],
            )
        slab = slabs.tile([KT, KQ * awid], mmdt, name=f"slab{gi}", tag="slab")
        nc.sync.dma_start(out=slab[:, :g * awid],
                          in_=a_t[gi * KT:(gi + 1) * KT, :g * awid])

        for sub in range(g):
            kt = k0 + sub
            xoff = kt * xwid
            base = sub * MSHARD if f8 else sub * awid
            for i, (c0, c1) in enumerate(MCH):
                # pass A: [x_hi | x_lo] (or plain x) against the A_hi half
                nc.tensor.matmul(
                    accs[i],
                    lhsT=xt[:, xoff:xoff + xwid],
                    rhs=slab[:, base + c0:base + c1],
                    start=(kt == 0),
                    stop=(kt == NKT - 1),
                )
            if split:
                for i, (c0, c1) in enumerate(MCH):
                    # pass B: x_hi (bf16) or x (fp8) against the A_lo half
                    if f8:
                        off = g * MSHARD + (sub * MSHARD + c0) // 2
                        rhs = slab[:, off:off + (c1 - c0) // 2].bitcast(fp8)
                        lo_lhs = xt8[:, kt * DIM:(kt + 1) * DIM]
                    else:
                        rhs = slab[:, base + MSHARD + c0:base + MSHARD + c1]
                        lo_lhs = xt[:, xoff:xoff + DIM]
                    nc.tensor.matmul(
                        accs_lo[i],
                        lhsT=lo_lhs,
                        rhs=rhs,
                        start=(kt == 0),
                        stop=(kt == NKT - 1),
                    )

    # epilogue-only constants: issued after the slab stream in program
    # order so they don't delay the first matmuls; they transfer during
    # the main loop and are ready long before the epilogue needs them.
    xs = consts.tile([MT, NMT * DIM], f32)
    nc.gpsimd.dma_start(out=xs, in_=xs_d)
    xst = consts.tile([DIM, MSHARD], f32)
    nc.gpsimd.dma_start(out=xst, in_=xst_d)
    ident = consts.tile([DIM, DIM], f32)
    nc.gpsimd.dma_start(out=ident, in_=id_d)

    # P = x^T * Ax^T  (elementwise), [64, 1250] in SBUF
    p_full = epil.tile([DIM, MSHARD], f32, bufs=1)
    for i, (c0, c1) in enumerate(MCH):
        w = c1 - c0
        if split:
            # only one PSUM operand allowed per DVE op -> chain via SBUF
            tsum = epil.tile([DIM, w], f32, name=f"tsum{i}", tag="tsum")
            nc.vector.tensor_copy(tsum, accs[i][0:DIM, :])
            nc.vector.tensor_add(tsum, tsum, accs[i][DIM:2 * DIM, :])
            if f8:
                tlo = epil.tile([DIM, w], f32, name=f"tlo{i}", tag="tlo")
                nc.vector.tensor_scalar(
                    out=tlo, in0=accs_lo[i], scalar1=1.0 / A_LO_SCALE,
                    scalar2=None, op0=mybir.AluOpType.mult)
                nc.vector.tensor_add(tsum, tsum, tlo)
            else:
                nc.vector.tensor_add(tsum, tsum, accs_lo[i])
            nc.vector.tensor_mul(p_full[:, c0:c1], xst[:, c0:c1], tsum)
        else:
            nc.vector.tensor_mul(p_full[:, c0:c1], xst[:, c0:c1], accs[i])

    for mt in range(NMT):
        pt = ptp.tile([MT, DIM], f32, name=f"pt{mt}", tag="pt")
        nc.tensor.transpose(
            out=pt, in_=p_full[:, mt * MT:(mt + 1) * MT], identity=ident,
        )
        s = epil.tile([MT, 1], f32, name=f"s{mt}", tag="s")
        nc.vector.tensor_reduce(
            out=s, in_=pt, axis=mybir.AxisListType.X, op=mybir.AluOpType.add,
        )
        t_col = epil.tile([MT, 1], f32, name=f"t{mt}", tag="t")
        # t = s * (-r) + F
        nc.vector.tensor_scalar(
            out=t_col, in0=s, scalar1=-R_CONST, scalar2=F_CONST,
            op0=mybir.AluOpType.mult, op1=mybir.AluOpType.add,
        )
        o = epil.tile([MT, DIM], f32, name=f"o{mt}", tag="o")
        nc.vector.tensor_scalar(
            out=o, in0=xs[:, mt * DIM:(mt + 1) * DIM], scalar1=bcol,
            scalar2=t_col, op0=mybir.AluOpType.mult, op1=mybir.AluOpType.add,
        )
        nc.sync.dma_start(out=out_d[mt * MT:(mt + 1) * MT, :], in_=o)


def build(layout=None, mm_dtype=None):
    layout = layout or DEFAULT_LAYOUT
    mm_dtype = mm_dtype or DEFAULT_MM_DTYPE
    key = (layout, mm_dtype)
    if key in _nc_cache:
        return _nc_cache[key]

    from contextlib import ExitStack
    import concourse.tile as tile
    from concourse import bacc

    mmdt, f32 = _dtypes(mm_dtype)

    nc = bacc.Bacc(
        "TRN2",
        target_bir_lowering=False,
        debug=False,
        enable_asserts=False,
        num_devices=NCORES,
        name=f"biochem_{layout}_{mm_dtype}",
    )
    from concourse import mybir

    if mm_dtype == "f8dr":
        a_t = nc.dram_tensor(
            "a_t", [NG_F8 * KT, KQ_F8, MSHARD], mmdt,
            kind="ExternalInput").ap()
        xt_d = nc.dram_tensor(
            "xt", [KT, NKT, DIM], mmdt, kind="ExternalInput").ap()
        xs_d = nc.dram_tensor(
            "xs", [MT, NMT * DIM], f32, kind="ExternalInput").ap()
        xst_d = nc.dram_tensor(
            "xst", [DIM, MSHARD], f32, kind="ExternalInput").ap()
        id_d = nc.dram_tensor(
            "ident", [DIM, DIM], f32, kind="ExternalInput").ap()
        out_d = nc.dram_tensor(
            "out", [MSHARD, DIM], f32, kind="ExternalOutput").ap()

        with tile.TileContext(nc) as tc:
            with ExitStack() as ctx:
                _body_f8dr(ctx, tc, a_t, xt_d, xs_d, xst_d, id_d, out_d)
        nc.compile()
        _nc_cache[key] = nc
        return nc

    split = mm_dtype in ("split", "splitf8")
    f8 = mm_dtype == "splitf8"
    awid = 2 * MSHARD if (split and not f8) else MSHARD
    if f8:
        awid = MSHARD + MSHARD // 2  # byte-packed hi(bf16)+lo(fp8)
    xwid = 2 * DIM if split else DIM
    # a_t is pre-tiled host-side into slab layout: row gi*128+p holds the
    # p-th partition of DMA group gi ([KQ consecutive k-rows] worth of data)
    a_t = nc.dram_tensor(
        "a_t", [NG * KT, KQ * awid], mmdt, kind="ExternalInput").ap()
    a_l = xt8_d = None
    if f8:
        xt8_d = nc.dram_tensor(
            "xt8", [KT, NKT * DIM], mybir.dt.float8e4,
            kind="ExternalInput").ap()
    xt_d = nc.dram_tensor("xt", [KT, NKT * xwid], mmdt, kind="ExternalInput").ap()
    xs_d = nc.dram_tensor("xs", [MT, NMT * DIM], f32, kind="ExternalInput").ap()
    if layout == "x_stat":
        xst_d = nc.dram_tensor("xst", [DIM, MSHARD], f32, kind="ExternalInput").ap()
        id_d = nc.dram_tensor("ident", [DIM, DIM], f32, kind="ExternalInput").ap()
    out_d = nc.dram_tensor("out", [MSHARD, DIM], f32, kind="ExternalOutput").ap()

    with tile.TileContext(nc) as tc:
        with ExitStack() as ctx:
            if layout == "x_stat":
                _body_x_stat(ctx, tc, a_t, a_l, xt_d, xt8_d, xs_d, xst_d,
                             id_d, out_d, mmdt, mm_dtype)
            else:
                raise ValueError(layout)
    nc.compile()
    _nc_cache[key] = nc
    return nc


def prepare_in_maps(x, A, layout=None, mm_dtype=None):
    layout = layout or DEFAULT_LAYOUT
    mm_dtype = mm_dtype or DEFAULT_MM_DTYPE
    np_mm = _np_mm_dtype(mm_dtype)

    x = np.asarray(x, np.float32)
    A = np.asarray(A, np.float32)

    if mm_dtype == "f8dr":
        ident = np.eye(DIM, dtype=np.float32)
        xp = np.zeros((KPAD, DIM), np.float32)
        xp[:N] = x
        xt_np = np.ascontiguousarray(
            xp.reshape(NKT, KT, DIM).transpose(1, 0, 2)).astype(np_mm)
        in_maps = []
        for c in range(NCORES):
            sh = slice(c * MSHARD, (c + 1) * MSHARD)
            at = np.zeros((KPAD, MSHARD), np.float32)
            at[:N] = A[sh].T
            at8 = at.astype(np_mm)
            a_t_c = np.zeros((NG_F8 * KT, KQ_F8, MSHARD), np_mm)
            for gi, (k0, g) in enumerate(KGROUPS_F8):
                blk = at8[k0 * KT:(k0 + g) * KT]
                a_t_c[gi * KT:(gi + 1) * KT, :g, :] = (
                    blk.reshape(g, KT, MSHARD).transpose(1, 0, 2))
            xs_c = np.ascontiguousarray(
                x[sh].reshape(NMT, MT, DIM).transpose(1, 0, 2)
                .reshape(MT, NMT * DIM))
            in_maps.append({
                "a_t": a_t_c, "xt": xt_np, "xs": xs_c,
                "xst": np.ascontiguousarray(x[sh].T), "ident": ident,
            })
        return in_maps

    split = mm_dtype in ("split", "splitf8")
    f8 = mm_dtype == "splitf8"
    if f8:
        import ml_dtypes
        np_fp8 = np.dtype(ml_dtypes.float8_e4m3)

    def tile_k(arr):
        """[KPAD, W] -> [KT, NKT*W] SBUF layout, padded rows are zero."""
        w = arr.shape[1]
        xp = np.zeros((KPAD, w), arr.dtype)
        xp[:N] = arr
        return np.ascontiguousarray(
            xp.reshape(NKT, KT, w).transpose(1, 0, 2).reshape(KT, NKT * w)
        )

    xt8_np = None
    if split:
        x_hi = x.astype(np_mm)
        x_lo = (x - x_hi.astype(np.float32)).astype(np_mm)
        # per k-tile stationary block is [x_hi | x_lo], 128 wide
        xt_np = tile_k(np.concatenate([x_hi, x_lo], axis=1))
        if f8:
            xt8_np = tile_k(x.astype(np_fp8))
    else:
        xt_np = tile_k(x).astype(np_mm)

    ident = np.eye(DIM, dtype=np.float32)

    def tile_slabs(at):
        """[KPAD, W] -> [NG*128, KQ*W] host pre-tiling into slab layout:
        row gi*128+p, cols sub*W:(sub+1)*W  =  at[(k0+sub)*128 + p, :]
        for group gi=(k0, g); unused columns of small groups stay zero."""
        w = at.shape[1]
        out = np.zeros((NG * KT, KQ * w), at.dtype)
        for gi, (k0, g) in enumerate(KGROUPS):
            blk = at[k0 * KT:(k0 + g) * KT, :]
            out[gi * KT:(gi + 1) * KT, :g * w] = (
                blk.reshape(g, KT, w).transpose(1, 0, 2).reshape(KT, g * w)
            )
        return out

    def pad_k(at):
        out = np.zeros((KPAD, at.shape[1]), at.dtype)
        out[:N] = at
        return out

    in_maps = []
    for c in range(NCORES):
        sh = slice(c * MSHARD, (c + 1) * MSHARD)
        at_f32 = pad_k(np.ascontiguousarray(A[sh].T))
        if f8:
            a_hi = at_f32.astype(np_mm)
            a_lo = at_f32 - a_hi.astype(np.float32)
            hi_t = tile_slabs(a_hi)                                # bf16
            lo_t = tile_slabs((a_lo * A_LO_SCALE).astype(np_fp8))  # fp8
            # byte-pack: per group row block, [g*2500 B hi][g*1250 B lo]
            awid = MSHARD + MSHARD // 2
            a_t_c = np.zeros((NG * KT, KQ * awid), np_mm)
            ob = a_t_c.view(np.uint8)
            hb = hi_t.view(np.uint8)
            lb = lo_t.view(np.uint8)
            for gi, (k0, g) in enumerate(KGROUPS):
                r = slice(gi * KT, (gi + 1) * KT)
                ob[r, :g * 2 * MSHARD] = hb[r, :g * 2 * MSHARD]
                ob[r, g * 2 * MSHARD:g * 3 * MSHARD] = lb[r, :g * MSHARD]
        elif split:
            a_hi = at_f32.astype(np_mm)
            a_lo = (at_f32 - a_hi.astype(np.float32)).astype(np_mm)
            a_t_c = tile_slabs(np.concatenate([a_hi, a_lo], axis=1))
        else:
            a_t_c = tile_slabs(at_f32.astype(np_mm))
        xs_c = np.ascontiguousarray(
            x[sh].reshape(NMT, MT, DIM).transpose(1, 0, 2).reshape(MT, NMT * DIM)
        )
        m = {"a_t": a_t_c, "xt": xt_np, "xs": xs_c}
        if f8:
            m["xt8"] = xt8_np
        if layout == "x_stat":
            m["xst"] = np.ascontiguousarray(x[sh].T)
            m["ident"] = ident
        in_maps.append(m)
    return in_maps


def run(inputs, trace=False, layout=None, mm_dtype=None, **spmd_kwargs):
    """Returns (full_output [10000, 64] f32, BassKernelResults)."""
    from concourse.bass_utils import run_bass_kernel_spmd

    nc = build(layout, mm_dtype)
    in_maps = prepare_in_maps(inputs["x"], inputs["A"], layout, mm_dtype)
    res = run_bass_kernel_spmd(
        nc, in_maps, core_ids=list(range(NCORES)), trace=trace, **spmd_kwargs
    )
    out = np.concatenate([res.results[c]["out"] for c in range(NCORES)], axis=0)
    return out, res


def kernel(t=None, x=None, A=None):
    out, _ = run({"x": x, "A": A})
    return out



# revision 12
# speedup vs baseline: 2.2495x; 1.0883x over previous
"""Trainium2 Bass kernel for nn_BiochemicalDiffusion.

Computes  out = F - B*x - r * rowsum(x * (A @ x))  for A:[10000,10000] f32,
x:[10000,64] f32, across 8 NeuronCores.

Sharding (all done host-side in this file):
  - A is sharded row-wise: core c gets rows [c*1250, (c+1)*1250).
  - The shard is passed pre-transposed (A_shard^T, [10000, 1250]) so the PE
    can contract over k directly: Ax_shard = A_shard^T.T @ x.
  - x is passed in full to every core (it is tiny), pre-tiled into the
    [128, 79*64] SBUF layout the matmul consumes.
  - Each core computes its [1250, 64] slice of the output; the host
    concatenates them.

Hardware note: PSUM accumulation groups must not share a PSUM bank — two
interleaved accumulation groups in one bank corrupt each other.  Both
layouts below keep one live accumulation group per bank.

Everything is hardcoded to the problem shapes; kernel.py is self-contained.
"""

import numpy as np

N = 10000
DIM = 64
NCORES = 8
MSHARD = N // NCORES  # 1250 rows of A / out per core
MT = 125              # m-tile (PSUM partition) size
NMT = MSHARD // MT    # 10 m-tiles per core
KT = 128              # k-tile (contraction) size
NKT = 79              # k-tiles covering the 10000 rows (last is 16+zeros)
KPAD = NKT * KT       # 10112 (rows 10000+ are zeros; they contribute 0)

F_CONST = 1.0
B_CONST = 0.1
R_CONST = 0.01

# m-chunks for the x-stationary layout (moving free dim >= 256 keeps fp32r
# at full rate; each chunk's accumulator owns one PSUM bank; widths must be
# EVEN -- fp32r matmul ISA restriction on innermost free counts)
MCH = [(0, 418), (418, 834), (834, 1250)]

# k-tile DMA groups: up to 4 k-tiles per transfer (~1.3-2.6 MB).  A^T is
# pre-tiled on the HOST into exactly this slab layout (group-major,
# partition-major inside a group) so each group is ONE flat contiguous
# 2D DMA -- large per-partition bursts, minimal descriptor work.  The
# first groups are deliberately small so the first matmul starts early
# (pipeline ramp), the steady state uses full quads.
KQ = 4
KGROUPS = ([(0, 1), (1, 1), (2, 2)]
           + [(k0, 4) for k0 in range(4, 76, 4)]
           + [(76, 3)])
NG = len(KGROUPS)                     # 22 groups covering 79 tiles

A_LO_SCALE = 512.0  # fp8 A_lo is stored pre-scaled into [-1, 1]

# f8dr variant: A entirely fp8e4 (1 B/elem of HBM traffic), matmuls in
# DoubleRow perf mode (2 k-tiles = 256 contraction rows per instruction at
# 0.5 cycles per output row).  k-tiles 0..77 ride in pairs; tile 78 (the
# 16-real-rows remainder) is a trailing single-row matmul.  Groups must be
# even-aligned and even-sized so pairs never straddle a slab boundary.
KQ_F8 = 8
KGROUPS_F8 = [(0, 2), (2, 2), (4, 4), (8, 6), (14, 8), (22, 8), (30, 8),
              (38, 8), (46, 8), (54, 8), (62, 8), (70, 8), (78, 1)]
NG_F8 = len(KGROUPS_F8)
SPLIT_GI = 8  # groups [0, SPLIT_GI) accumulate into half A, rest into half B
assert sum(g for _, g in KGROUPS_F8) == NKT
NTAIL = N - (NKT - 1) * KT  # 16 real k-rows in the last k-tile

DEFAULT_LAYOUT = "x_stat"    # only x_stat is implemented
DEFAULT_MM_DTYPE = "f8dr"    # "f32r" | "bf16" | "split" | "splitf8" | "f8dr"

_nc_cache = {}


def _dtypes(mm_dtype):
    from concourse import mybir
    mm = {
        "f32": mybir.dt.float32,
        "f32r": mybir.dt.float32r,
        "bf16": mybir.dt.bfloat16,
        "split": mybir.dt.bfloat16,
        "splitf8": mybir.dt.bfloat16,
        "f8dr": mybir.dt.float8e4,
    }[mm_dtype]
    return mm, mybir.dt.float32


def _np_mm_dtype(mm_dtype):
    if mm_dtype in ("bf16", "split", "splitf8"):
        import ml_dtypes
        return np.dtype(ml_dtypes.bfloat16)
    if mm_dtype == "f8dr":
        import ml_dtypes
        return np.dtype(ml_dtypes.float8_e4m3)
    return np.dtype(np.float32)


def _body_f8dr(ctx, tc, a_t, a_tail, xt_d, xst_d, id_d, out_d):
    """All-fp8 A stream with DoubleRow matmuls and split-k epilogue overlap.

    Per slab group: one contiguous fp8 DMA ([128, g, 1250], up to 10 KB per
    partition line) on the sync HWDGE queue, then g/2 DoubleRow matmuls per
    m-chunk, each consuming 2 k-tiles (256 contraction rows) of both the
    stationary x and the moving A^T slab via 3D APs [128, 2, w].  The
    remainder k-tile 78 (rows 9984..10000) is a 16-partition single-row
    matmul from its own tiny dram tensor.

    k is split in two halves with separate PSUM accumulator sets (6 banks);
    half A's epilogue (p = x^T*Ax^T, transpose, rowsum) runs on DVE/PE while
    half B's slabs still stream, so only half B's epilogue is tail-serial.
    x in [125, 640] layout is derived on-chip from x^T via PE transposes
    instead of a separate DMA.  All non-A traffic rides the scalar HWDGE
    queue; gpsimd (software DGE, slow drains) is unused."""
    import concourse.bass  # noqa: F401
    from concourse import mybir

    nc = tc.nc
    f32 = mybir.dt.float32
    fp8 = mybir.dt.float8e4
    DR = mybir.MatmulPerfMode.DoubleRow

    consts = ctx.enter_context(tc.tile_pool(name="consts", bufs=1))
    slabs = ctx.enter_context(tc.tile_pool(name="slabs", bufs=6))
    psums = ctx.enter_context(tc.tile_pool(name="psums", bufs=1, space="PSUM"))
    ptp = ctx.enter_context(tc.tile_pool(name="ptp", bufs=2, space="PSUM"))
    epil = ctx.enter_context(tc.tile_pool(name="epil", bufs=2))

    # scalar-queue constants; xst first so the prologue transposes can start
    xst = consts.tile([DIM, MSHARD], f32)
    nc.scalar.dma_start(out=xst, in_=xst_d)
    ident = consts.tile([DIM, DIM], f32)
    nc.scalar.dma_start(out=ident, in_=id_d)
    xt = consts.tile([KT, NKT, DIM], fp8)
    nc.scalar.dma_start(out=xt[:, 0:2, :], in_=xt_d[:, 0:2, :])

    bcol = consts.tile([MT, 1], f32)
    nc.vector.memset(bcol, -B_CONST)
    xs = consts.tile([MT, NMT * DIM], f32)
    s_full = consts.tile([MT, 2 * NMT], f32)

    acc_ab = [
        [psums.tile([DIM, c1 - c0], f32, name=f"acc{h}{i}", tag=f"acc{h}{i}")
         for i, (c0, c1) in enumerate(MCH)]
        for h in range(2)
    ]
    SPLIT_KT = KGROUPS_F8[SPLIT_GI][0]
    STOP_A = KGROUPS_F8[SPLIT_GI][0] - 2  # first kt of the last A pair

    p_ab = [None, None]

    def half_mul(h):
        # p = x^T * Ax^T elementwise (DVE, reads the stopped accumulators)
        p = epil.tile([DIM, MSHARD], f32, name=f"p{h}", tag=f"p{h}", bufs=1)
        p_ab[h] = p
        for i, (c0, c1) in enumerate(MCH):
            nc.vector.tensor_mul(p[:, c0:c1], xst[:, c0:c1], acc_ab[h][i])

    def half_reduce(h):
        # per m-tile PE transpose + rowsum into s_full[:, h*NMT + mt]
        p = p_ab[h]
        for mt in range(NMT):
            pt = ptp.tile([MT, DIM], f32, name=f"pt{h}{mt}", tag="pt")
            nc.tensor.transpose(
                out=pt, in_=p[:, mt * MT:(mt + 1) * MT], identity=ident,
            )
            col = h * NMT + mt
            nc.vector.tensor_reduce(
                out=s_full[:, col:col + 1], in_=pt,
                axis=mybir.AxisListType.X, op=mybir.AluOpType.add,
            )

    for gi, (k0, g) in enumerate(KGROUPS_F8):
        h = 0 if gi < SPLIT_GI else 1
        accs = acc_ab[h]
        if g == 1:  # 16-partition remainder tile (half B's stop)
            nc.scalar.dma_start(out=xt[:, NKT - 1:NKT, :],
                                in_=xt_d[:, NKT - 1:NKT, :])
            slab_t = consts.tile([NTAIL, MSHARD], fp8)
            nc.scalar.dma_start(out=slab_t, in_=a_tail)
            for i, (c0, c1) in enumerate(MCH):
                nc.tensor.matmul(
                    accs[i],
                    lhsT=xt[0:16, NKT - 1, :],
                    rhs=slab_t[:, c0:c1],
                    start=False,
                    stop=True,
                )
            continue
        if gi > 0:
            nc.scalar.dma_start(out=xt[:, k0:k0 + g, :],
                                in_=xt_d[:, k0:k0 + g, :])
        slab = slabs.tile([KT, KQ_F8, MSHARD], fp8, name=f"slab{gi}",
                          tag="slab")
        nc.sync.dma_start(out=slab[:, :g, :],
                          in_=a_t[gi * KT:(gi + 1) * KT, :g, :])
        for sub in range(0, g - 1, 2):
            kt = k0 + sub
            for i, (c0, c1) in enumerate(MCH):
                nc.tensor.matmul(
                    accs[i],
                    lhsT=xt[:, kt:kt + 2, :],
                    rhs=slab[:, sub:sub + 2, c0:c1],
                    start=(kt == 0 or kt == SPLIT_KT),
                    stop=(kt == STOP_A and h == 0),
                    perf_mode=DR,
                )
        if gi == 1:
            # derive xs ([125, 640] x-shard) from xst on otherwise-idle
            # engines during the DMA ramp
            for mt in range(NMT):
                px = ptp.tile([MT, DIM], f32, name=f"px{mt}", tag="pt")
                nc.tensor.transpose(
                    out=px, in_=xst[:, mt * MT:(mt + 1) * MT], identity=ident,
                )
                nc.vector.tensor_copy(xs[:, mt * DIM:(mt + 1) * DIM], px)
        if gi == SPLIT_GI - 1:
            half_mul(0)          # DVE: overlaps half B's matmuls
        if gi == SPLIT_GI:
            half_reduce(0)       # PE/DVE: p0 is ready by now

    half_mul(1)
    half_reduce(1)

    # t = (s_a + s_b) * (-r) + F, then o = x * (-b) + t per m-tile
    ssum = epil.tile([MT, NMT], f32, bufs=1)
    nc.vector.tensor_add(ssum, s_full[:, :NMT], s_full[:, NMT:])
    t_full = epil.tile([MT, NMT], f32, bufs=1)
    nc.vector.tensor_scalar(
        out=t_full, in0=ssum, scalar1=-R_CONST, scalar2=F_CONST,
        op0=mybir.AluOpType.mult, op1=mybir.AluOpType.add,
    )
    for mt in range(NMT):
        o = epil.tile([MT, DIM], f32, name=f"o{mt}", tag="o")
        nc.vector.tensor_scalar(
            out=o, in0=xs[:, mt * DIM:(mt + 1) * DIM], scalar1=bcol,
            scalar2=t_full[:, mt:mt + 1], op0=mybir.AluOpType.mult,
            op1=mybir.AluOpType.add,
        )
        nc.scalar.dma_start(out=out_d[mt * MT:(mt + 1) * MT, :], in_=o)


def _body_x_stat(ctx, tc, a_t, a_l, xt_d, xt8_d, xs_d, xst_d, id_d, out_d,
                 mmdt, mm_dtype):
    """k-outer loop; x k-tiles are the stationary operand, A^T slabs stream
    as the moving operand (large free dim -> full-rate fp32r / bf16).
    Produces Ax^T in PSUM (3 chunk accumulators, one bank each); epilogue
    transposes x^T*Ax^T back via the PE.

    DMA streams in KQ-k-tile groups (~1.3-2.6 MB per transfer) to amortize
    per-DMA overhead; the stationary x is preloaded in per-group chunks on
    the gpsimd queue so the first matmul does not wait for the whole x.

    split: A and x decomposed as hi+lo bf16 pairs; A@x ~= A_hi@x_hi +
    A_lo@x_hi + A_hi@x_lo.  a_t holds [A_hi^T | A_lo^T] side by side; xt
    holds [x_hi | x_lo] per k-tile so the two x terms ride in ONE 128-wide
    stationary: pass A computes both x_hi@A_hi (psum rows 0:64) and
    x_lo@A_hi (rows 64:128) in a single moving sweep of the A_hi slab
    half; pass B computes x_hi@A_lo.

    splitf8: like split but A_lo is a SEPARATE fp8e4m3 tensor pre-scaled
    by A_LO_SCALE, and pass B runs all-fp8 (x in fp8) -- 3 bytes/element
    of A traffic instead of 4; epilogue rescales the pass-B accumulator."""
    import concourse.bass  # noqa: F401
    from concourse import mybir

    nc = tc.nc
    f32 = mybir.dt.float32
    fp8 = mybir.dt.float8e4
    split = mm_dtype in ("split", "splitf8")
    f8 = mm_dtype == "splitf8"

    consts = ctx.enter_context(tc.tile_pool(name="consts", bufs=1))
    slabs = ctx.enter_context(tc.tile_pool(name="slabs", bufs=6))
    psums = ctx.enter_context(tc.tile_pool(name="psums", bufs=1, space="PSUM"))
    ptp = ctx.enter_context(tc.tile_pool(name="ptp", bufs=2, space="PSUM"))
    epil = ctx.enter_context(tc.tile_pool(name="epil", bufs=2))

    # elements per k-row in the a_t tensor.  For splitf8 the hi (bf16) and
    # lo (fp8) halves are byte-packed into one bf16-typed stream:
    # per k-tile per partition = 1250 bf16 hi elems then 1250 fp8 lo bytes
    # (= 625 bf16-elem slots); pass B reads the lo region via bitcast.
    awid = 2 * MSHARD if (split and not f8) else MSHARD
    if f8:
        awid = MSHARD + MSHARD // 2  # 1875 bf16 elems per k-tile
    xwid = 2 * DIM if split else DIM  # stationary block width per k-tile

    xt = consts.tile([KT, NKT * xwid], mmdt)
    if f8:
        xt8 = consts.tile([KT, NKT * DIM], fp8)
    bcol = consts.tile([MT, 1], f32)
    nc.vector.memset(bcol, -B_CONST)

    accs = [psums.tile([xwid, c1 - c0], f32, name=f"acc{i}", tag=f"acc{i}")
            for i, (c0, c1) in enumerate(MCH)]
    if split:
        accs_lo = [psums.tile([DIM, c1 - c0], f32, name=f"accl{i}",
                              tag=f"accl{i}")
                   for i, (c0, c1) in enumerate(MCH)]

    for gi, (k0, g) in enumerate(KGROUPS):
        # stationary chunk for this group's k-tiles (gpsimd queue, overlaps
        # with the slab stream on the sync queue)
        nc.gpsimd.dma_start(
            out=xt[:, k0 * xwid:(k0 + g) * xwid],
            in_=xt_d[:, k0 * xwid:(k0 + g) * xwid],
        )
        if f8:
            nc.gpsimd.dma_start(
                out=xt8[:, k0 * DIM:(k0 + g) * DIM],
                in_=xt8_d[:, k0 * DIM:(k0 + g) * DIM],
            )
        slab = slabs.tile([KT, KQ * awid], mmdt, name=f"slab{gi}", tag="slab")
        nc.sync.dma_start(out=slab[:, :g * awid],
                          in_=a_t[gi * KT:(gi + 1) * KT, :g * awid])

        for sub in range(g):
            kt = k0 + sub
            xoff = kt * xwid
            base = sub * MSHARD if f8 else sub * awid
            for i, (c0, c1) in enumerate(MCH):
                # pass A: [x_hi | x_lo] (or plain x) against the A_hi half
                nc.tensor.matmul(
                    accs[i],
                    lhsT=xt[:, xoff:xoff + xwid],
                    rhs=slab[:, base + c0:base + c1],
                    start=(kt == 0),
                    stop=(kt == NKT - 1),
                )
            if split:
                for i, (c0, c1) in enumerate(MCH):
                    # pass B: x_hi (bf16) or x (fp8) against the A_lo half
                    if f8:
                        off = g * MSHARD + (sub * MSHARD + c0) // 2
                        rhs = slab[:, off:off + (c1 - c0) // 2].bitcast(fp8)
                        lo_lhs = xt8[:, kt * DIM:(kt + 1) * DIM]
                    else:
                        rhs = slab[:, base + MSHARD + c0:base + MSHARD + c1]
                        lo_lhs = xt[:, xoff:xoff + DIM]
                    nc.tensor.matmul(
                        accs_lo[i],
                        lhsT=lo_lhs,
                        rhs=rhs,
                        start=(kt == 0),
                        stop=(kt == NKT - 1),
                    )

    # epilogue-only constants: issued after the slab stream in program
    # order so they don't delay the first matmuls; they transfer during
    # the main loop and are ready long before the epilogue needs them.
    xs = consts.tile([MT, NMT * DIM], f32)
    nc.gpsimd.dma_start(out=xs, in_=xs_d)
    xst = consts.tile([DIM, MSHARD], f32)
    nc.gpsimd.dma_start(out=xst, in_=xst_d)
    ident = consts.tile([DIM, DIM], f32)
    nc.gpsimd.dma_start(out=ident, in_=id_d)

    # P = x^T * Ax^T  (elementwise), [64, 1250] in SBUF
    p_full = epil.tile([DIM, MSHARD], f32, bufs=1)
    for i, (c0, c1) in enumerate(MCH):
        w = c1 - c0
        if split:
            # only one PSUM operand allowed per DVE op -> chain via SBUF
            tsum = epil.tile([DIM, w], f32, name=f"tsum{i}", tag="tsum")
            nc.vector.tensor_copy(tsum, accs[i][0:DIM, :])
            nc.vector.tensor_add(tsum, tsum, accs[i][DIM:2 * DIM, :])
            if f8:
                tlo = epil.tile([DIM, w], f32, name=f"tlo{i}", tag="tlo")
                nc.vector.tensor_scalar(
                    out=tlo, in0=accs_lo[i], scalar1=1.0 / A_LO_SCALE,
                    scalar2=None, op0=mybir.AluOpType.mult)
                nc.vector.tensor_add(tsum, tsum, tlo)
            else:
                nc.vector.tensor_add(tsum, tsum, accs_lo[i])
            nc.vector.tensor_mul(p_full[:, c0:c1], xst[:, c0:c1], tsum)
        else:
            nc.vector.tensor_mul(p_full[:, c0:c1], xst[:, c0:c1], accs[i])

    for mt in range(NMT):
        pt = ptp.tile([MT, DIM], f32, name=f"pt{mt}", tag="pt")
        nc.tensor.transpose(
            out=pt, in_=p_full[:, mt * MT:(mt + 1) * MT], identity=ident,
        )
        s = epil.tile([MT, 1], f32, name=f"s{mt}", tag="s")
        nc.vector.tensor_reduce(
            out=s, in_=pt, axis=mybir.AxisListType.X, op=mybir.AluOpType.add,
        )
        t_col = epil.tile([MT, 1], f32, name=f"t{mt}", tag="t")
        # t = s * (-r) + F
        nc.vector.tensor_scalar(
            out=t_col, in0=s, scalar1=-R_CONST, scalar2=F_CONST,
            op0=mybir.AluOpType.mult, op1=mybir.AluOpType.add,
        )
        o = epil.tile([MT, DIM], f32, name=f"o{mt}", tag="o")
        nc.vector.tensor_scalar(
            out=o, in0=xs[:, mt * DIM:(mt + 1) * DIM], scalar1=bcol,
            scalar2=t_col, op0=mybir.AluOpType.mult, op1=mybir.AluOpType.add,
        )
        nc.sync.dma_start(out=out_d[mt * MT:(mt + 1) * MT, :], in_=o)


def build(layout=None, mm_dtype=None):
    layout = layout or DEFAULT_LAYOUT
    mm_dtype = mm_dtype or DEFAULT_MM_DTYPE
    key = (layout, mm_dtype)
    if key in _nc_cache:
        return _nc_cache[key]

    from contextlib import ExitStack
    import concourse.tile as tile
    from concourse import bacc

    mmdt, f32 = _dtypes(mm_dtype)

    nc = bacc.Bacc(
        "TRN2",
        target_bir_lowering=False,
        debug=False,
        enable_asserts=False,
        num_devices=NCORES,
        name=f"biochem_{layout}_{mm_dtype}",
    )
    from concourse import mybir

    if mm_dtype == "f8dr":
        a_t = nc.dram_tensor(
            "a_t", [(NG_F8 - 1) * KT, KQ_F8, MSHARD], mmdt,
            kind="ExternalInput").ap()
        a_tail = nc.dram_tensor(
            "a_tail", [NTAIL, MSHARD], mmdt, kind="ExternalInput").ap()
        xt_d = nc.dram_tensor(
            "xt", [KT, NKT, DIM], mmdt, kind="ExternalInput").ap()
        xst_d = nc.dram_tensor(
            "xst", [DIM, MSHARD], f32, kind="ExternalInput").ap()
        id_d = nc.dram_tensor(
            "ident", [DIM, DIM], f32, kind="ExternalInput").ap()
        out_d = nc.dram_tensor(
            "out", [MSHARD, DIM], f32, kind="ExternalOutput").ap()

        with tile.TileContext(nc) as tc:
            with ExitStack() as ctx:
                _body_f8dr(ctx, tc, a_t, a_tail, xt_d, xst_d, id_d, out_d)
        nc.compile()
        _nc_cache[key] = nc
        return nc

    split = mm_dtype in ("split", "splitf8")
    f8 = mm_dtype == "splitf8"
    awid = 2 * MSHARD if (split and not f8) else MSHARD
    if f8:
        awid = MSHARD + MSHARD // 2  # byte-packed hi(bf16)+lo(fp8)
    xwid = 2 * DIM if split else DIM
    # a_t is pre-tiled host-side into slab layout: row gi*128+p holds the
    # p-th partition of DMA group gi ([KQ consecutive k-rows] worth of data)
    a_t = nc.dram_tensor(
        "a_t", [NG * KT, KQ * awid], mmdt, kind="ExternalInput").ap()
    a_l = xt8_d = None
    if f8:
        xt8_d = nc.dram_tensor(
            "xt8", [KT, NKT * DIM], mybir.dt.float8e4,
            kind="ExternalInput").ap()
    xt_d = nc.dram_tensor("xt", [KT, NKT * xwid], mmdt, kind="ExternalInput").ap()
    xs_d = nc.dram_tensor("xs", [MT, NMT * DIM], f32, kind="ExternalInput").ap()
    if layout == "x_stat":
        xst_d = nc.dram_tensor("xst", [DIM, MSHARD], f32, kind="ExternalInput").ap()
        id_d = nc.dram_tensor("ident", [DIM, DIM], f32, kind="ExternalInput").ap()
    out_d = nc.dram_tensor("out", [MSHARD, DIM], f32, kind="ExternalOutput").ap()

    with tile.TileContext(nc) as tc:
        with ExitStack() as ctx:
            if layout == "x_stat":
                _body_x_stat(ctx, tc, a_t, a_l, xt_d, xt8_d, xs_d, xst_d,
                             id_d, out_d, mmdt, mm_dtype)
            else:
                raise ValueError(layout)
    nc.compile()
    _nc_cache[key] = nc
    return nc


def prepare_in_maps(x, A, layout=None, mm_dtype=None):
    layout = layout or DEFAULT_LAYOUT
    mm_dtype = mm_dtype or DEFAULT_MM_DTYPE
    np_mm = _np_mm_dtype(mm_dtype)

    x = np.asarray(x, np.float32)
    A = np.asarray(A, np.float32)

    if mm_dtype == "f8dr":
        ident = np.eye(DIM, dtype=np.float32)
        xp = np.zeros((KPAD, DIM), np.float32)
        xp[:N] = x
        xt_np = np.ascontiguousarray(
            xp.reshape(NKT, KT, DIM).transpose(1, 0, 2)).astype(np_mm)
        in_maps = []
        for c in range(NCORES):
            sh = slice(c * MSHARD, (c + 1) * MSHARD)
            at8 = np.ascontiguousarray(A[sh].T).astype(np_mm)  # [N, MSHARD]
            a_t_c = np.zeros(((NG_F8 - 1) * KT, KQ_F8, MSHARD), np_mm)
            for gi, (k0, g) in enumerate(KGROUPS_F8[:-1]):
                blk = at8[k0 * KT:(k0 + g) * KT]
                a_t_c[gi * KT:(gi + 1) * KT, :g, :] = (
                    blk.reshape(g, KT, MSHARD).transpose(1, 0, 2))
            in_maps.append({
                "a_t": a_t_c,
                "a_tail": np.ascontiguousarray(at8[(NKT - 1) * KT:]),
                "xt": xt_np,
                "xst": np.ascontiguousarray(x[sh].T), "ident": ident,
            })
        return in_maps

    split = mm_dtype in ("split", "splitf8")
    f8 = mm_dtype == "splitf8"
    if f8:
        import ml_dtypes
        np_fp8 = np.dtype(ml_dtypes.float8_e4m3)

    def tile_k(arr):
        """[KPAD, W] -> [KT, NKT*W] SBUF layout, padded rows are zero."""
        w = arr.shape[1]
        xp = np.zeros((KPAD, w), arr.dtype)
        xp[:N] = arr
        return np.ascontiguousarray(
            xp.reshape(NKT, KT, w).transpose(1, 0, 2).reshape(KT, NKT * w)
        )

    xt8_np = None
    if split:
        x_hi = x.astype(np_mm)
        x_lo = (x - x_hi.astype(np.float32)).astype(np_mm)
        # per k-tile stationary block is [x_hi | x_lo], 128 wide
        xt_np = tile_k(np.concatenate([x_hi, x_lo], axis=1))
        if f8:
            xt8_np = tile_k(x.astype(np_fp8))
    else:
        xt_np = tile_k(x).astype(np_mm)

    ident = np.eye(DIM, dtype=np.float32)

    def tile_slabs(at):
        """[KPAD, W] -> [NG*128, KQ*W] host pre-tiling into slab layout:
        row gi*128+p, cols sub*W:(sub+1)*W  =  at[(k0+sub)*128 + p, :]
        for group gi=(k0, g); unused columns of small groups stay zero."""
        w = at.shape[1]
        out = np.zeros((NG * KT, KQ * w), at.dtype)
        for gi, (k0, g) in enumerate(KGROUPS):
            blk = at[k0 * KT:(k0 + g) * KT, :]
            out[gi * KT:(gi + 1) * KT, :g * w] = (
                blk.reshape(g, KT, w).transpose(1, 0, 2).reshape(KT, g * w)
            )
        return out

    def pad_k(at):
        out = np.zeros((KPAD, at.shape[1]), at.dtype)
        out[:N] = at
        return out

    in_maps = []
    for c in range(NCORES):
        sh = slice(c * MSHARD, (c + 1) * MSHARD)
        at_f32 = pad_k(np.ascontiguousarray(A[sh].T))
        if f8:
            a_hi = at_f32.astype(np_mm)
            a_lo = at_f32 - a_hi.astype(np.float32)
            hi_t = tile_slabs(a_hi)                                # bf16
            lo_t = tile_slabs((a_lo * A_LO_SCALE).astype(np_fp8))  # fp8
            # byte-pack: per group row block, [g*2500 B hi][g*1250 B lo]
            awid = MSHARD + MSHARD // 2
            a_t_c = np.zeros((NG * KT, KQ * awid), np_mm)
            ob = a_t_c.view(np.uint8)
            hb = hi_t.view(np.uint8)
            lb = lo_t.view(np.uint8)
            for gi, (k0, g) in enumerate(KGROUPS):
                r = slice(gi * KT, (gi + 1) * KT)
                ob[r, :g * 2 * MSHARD] = hb[r, :g * 2 * MSHARD]
                ob[r, g * 2 * MSHARD:g * 3 * MSHARD] = lb[r, :g * MSHARD]
        elif split:
            a_hi = at_f32.astype(np_mm)
            a_lo = (at_f32 - a_hi.astype(np.float32)).astype(np_mm)
            a_t_c = tile_slabs(np.concatenate([a_hi, a_lo], axis=1))
        else:
            a_t_c = tile_slabs(at_f32.astype(np_mm))
        xs_c = np.ascontiguousarray(
            x[sh].reshape(NMT, MT, DIM).transpose(1, 0, 2).reshape(MT, NMT * DIM)
        )
        m = {"a_t": a_t_c, "xt": xt_np, "xs": xs_c}
        if f8:
            m["xt8"] = xt8_np
        if layout == "x_stat":
            m["xst"] = np.ascontiguousarray(x[sh].T)
            m["ident"] = ident
        in_maps.append(m)
    return in_maps


def run(inputs, trace=False, layout=None, mm_dtype=None, **spmd_kwargs):
    """Returns (full_output [10000, 64] f32, BassKernelResults)."""
    from concourse.bass_utils import run_bass_kernel_spmd

    nc = build(layout, mm_dtype)
    in_maps = prepare_in_maps(inputs["x"], inputs["A"], layout, mm_dtype)
    res = run_bass_kernel_spmd(
        nc, in_maps, core_ids=list(range(NCORES)), trace=trace, **spmd_kwargs
    )
    out = np.concatenate([res.results[c]["out"] for c in range(NCORES)], axis=0)
    return out, res


def kernel(t=None, x=None, A=None):
    out, _ = run({"x": x, "A": A})
    return out



# revision 19
# speedup vs baseline: 2.4213x; 1.0764x over previous
"""Trainium2 Bass kernel for nn_BiochemicalDiffusion.

Computes  out = F - B*x - r * rowsum(x * (A @ x))  for A:[10000,10000] f32,
x:[10000,64] f32, across 8 NeuronCores.

Sharding (all done host-side in this file):
  - A is sharded row-wise: core c gets rows [c*1250, (c+1)*1250).
  - The shard is passed pre-transposed (A_shard^T, [10000, 1250]) so the PE
    can contract over k directly: Ax_shard = A_shard^T.T @ x.
  - x is passed in full to every core (it is tiny), pre-tiled into the
    [128, 79*64] SBUF layout the matmul consumes.
  - Each core computes its [1250, 64] slice of the output; the host
    concatenates them.

Hardware note: PSUM accumulation groups must not share a PSUM bank — two
interleaved accumulation groups in one bank corrupt each other.  Both
layouts below keep one live accumulation group per bank.

Everything is hardcoded to the problem shapes; kernel.py is self-contained.
"""

import numpy as np

N = 10000
DIM = 64
NCORES = 8
MSHARD = N // NCORES  # 1250 rows of A / out per core
MT = 125              # m-tile (PSUM partition) size
NMT = MSHARD // MT    # 10 m-tiles per core
KT = 128              # k-tile (contraction) size
NKT = 79              # k-tiles covering the 10000 rows (last is 16+zeros)
KPAD = NKT * KT       # 10112 (rows 10000+ are zeros; they contribute 0)

F_CONST = 1.0
B_CONST = 0.1
R_CONST = 0.01

# m-chunks for the x-stationary layout (moving free dim >= 256 keeps fp32r
# at full rate; each chunk's accumulator owns one PSUM bank; widths must be
# EVEN -- fp32r matmul ISA restriction on innermost free counts)
MCH = [(0, 418), (418, 834), (834, 1250)]

# k-tile DMA groups: up to 4 k-tiles per transfer (~1.3-2.6 MB).  A^T is
# pre-tiled on the HOST into exactly this slab layout (group-major,
# partition-major inside a group) so each group is ONE flat contiguous
# 2D DMA -- large per-partition bursts, minimal descriptor work.  The
# first groups are deliberately small so the first matmul starts early
# (pipeline ramp), the steady state uses full quads.
KQ = 4
KGROUPS = ([(0, 1), (1, 1), (2, 2)]
           + [(k0, 4) for k0 in range(4, 76, 4)]
           + [(76, 3)])
NG = len(KGROUPS)                     # 22 groups covering 79 tiles

A_LO_SCALE = 512.0  # fp8 A_lo is stored pre-scaled into [-1, 1]

# f8dr variant: A entirely fp8e4 (1 B/elem of HBM traffic), matmuls in
# DoubleRow perf mode (2 k-tiles = 256 contraction rows per instruction at
# 0.5 cycles per output row).  k-tiles 0..77 ride in pairs; tile 78 (the
# 16-real-rows remainder) is a trailing single-row matmul.  Groups must be
# even-aligned and even-sized so pairs never straddle a slab boundary.
KQ_F8 = 8
KGROUPS_F8 = [(0, 2), (2, 2), (4, 4), (8, 6), (14, 8), (22, 8), (30, 8),
              (38, 8), (46, 8), (54, 8), (62, 8), (70, 8), (78, 1)]
NG_F8 = len(KGROUPS_F8)
SPLIT_GI = 8  # groups [0, SPLIT_GI) accumulate into half A, rest into half B
assert sum(g for _, g in KGROUPS_F8) == NKT
NTAIL = N - (NKT - 1) * KT  # 16 real k-rows in the last k-tile

DEFAULT_LAYOUT = "x_stat"    # only x_stat is implemented
DEFAULT_MM_DTYPE = "f8dr"    # "f32r" | "bf16" | "split" | "splitf8" | "f8dr"

_nc_cache = {}


def _dtypes(mm_dtype):
    from concourse import mybir
    mm = {
        "f32": mybir.dt.float32,
        "f32r": mybir.dt.float32r,
        "bf16": mybir.dt.bfloat16,
        "split": mybir.dt.bfloat16,
        "splitf8": mybir.dt.bfloat16,
        "f8dr": mybir.dt.float8e4,
    }[mm_dtype]
    return mm, mybir.dt.float32


def _np_mm_dtype(mm_dtype):
    if mm_dtype in ("bf16", "split", "splitf8"):
        import ml_dtypes
        return np.dtype(ml_dtypes.bfloat16)
    if mm_dtype == "f8dr":
        import ml_dtypes
        return np.dtype(ml_dtypes.float8_e4m3)
    return np.dtype(np.float32)


def _body_f8dr(ctx, tc, a_t, a_tail, xt_d, xs_d, xst_d, out_d):
    """All-fp8 A stream with DoubleRow matmuls and split-k epilogue overlap.

    Per slab group: one contiguous fp8 DMA ([128, g, 1250], up to 10 KB per
    partition line) on the sync HWDGE queue, then g/2 DoubleRow matmuls per
    m-chunk, each consuming 2 k-tiles (256 contraction rows) of both the
    stationary x and the moving A^T slab via 3D APs [128, 2, w].  The
    remainder k-tile 78 (rows 9984..10000) is a 16-partition single-row
    matmul from its own tiny dram tensor.

    k is split in two halves with separate PSUM accumulator sets (6 banks);
    half A's epilogue (p = x^T*Ax^T, transpose, rowsum) runs on DVE/PE while
    half B's slabs still stream, so only half B's epilogue is tail-serial.
    x in [125, 640] layout is derived on-chip from x^T via PE transposes
    instead of a separate DMA.  All non-A traffic rides the scalar HWDGE
    queue; gpsimd (software DGE, slow drains) is unused."""
    import concourse.bass  # noqa: F401
    from concourse import mybir

    nc = tc.nc
    f32 = mybir.dt.float32
    fp8 = mybir.dt.float8e4
    DR = mybir.MatmulPerfMode.DoubleRow

    consts = ctx.enter_context(tc.tile_pool(name="consts", bufs=1))
    slabs = ctx.enter_context(tc.tile_pool(name="slabs", bufs=8))
    psums = ctx.enter_context(tc.tile_pool(name="psums", bufs=1, space="PSUM"))
    epil = ctx.enter_context(tc.tile_pool(name="epil", bufs=2))

    # scalar-queue constants
    xst = consts.tile([DIM, MSHARD], f32)
    nc.scalar.dma_start(out=xst, in_=xst_d)
    xs = consts.tile([MT, NMT * DIM], f32)
    nc.scalar.dma_start(out=xs, in_=xs_d)
    xt = consts.tile([KT, NKT, DIM], fp8)
    nc.scalar.dma_start(out=xt[:, 0:2, :], in_=xt_d[:, 0:2, :])

    bcol = consts.tile([MT, 1], f32)
    nc.vector.memset(bcol, -B_CONST)
    ones = consts.tile([DIM, 1], f32)
    nc.vector.memset(ones, 1.0)

    acc_ab = [
        [psums.tile([DIM, c1 - c0], f32, name=f"acc{h}{i}", tag=f"acc{h}{i}")
         for i, (c0, c1) in enumerate(MCH)]
        for h in range(2)
    ]
    # s = rowsum(x^T * Ax^T) columns land here via ones-matmuls; 20
    # single-shot groups, strictly sequential, one bank
    s_ps = psums.tile([MT, 2 * NMT], f32)
    SPLIT_KT = KGROUPS_F8[SPLIT_GI][0]
    STOP_A = KGROUPS_F8[SPLIT_GI][0] - 2  # first kt of the last A pair

    p_ab = [None, None]
    sa_s = consts.tile([MT, NMT], f32)

    def half_mul(h):
        # p = x^T * Ax^T elementwise (DVE, reads the stopped accumulators)
        p = epil.tile([DIM, MSHARD], f32, name=f"p{h}", tag=f"p{h}", bufs=1)
        p_ab[h] = p
        for i, (c0, c1) in enumerate(MCH):
            nc.vector.tensor_mul(p[:, c0:c1], xst[:, c0:c1], acc_ab[h][i])

    def half_reduce(h):
        # s[:, mt] = p[:, mt-block]^T @ ones  (PE rowsum, no transposes)
        p = p_ab[h]
        for mt in range(NMT):
            col = h * NMT + mt
            nc.tensor.matmul(
                s_ps[:, col:col + 1],
                lhsT=p[:, mt * MT:(mt + 1) * MT],
                rhs=ones,
                start=True,
                stop=True,
            )
        if h == 0:
            # park half A's sums in SBUF so the final add has only one
            # PSUM operand
            nc.vector.tensor_copy(sa_s, s_ps[:, :NMT])

    for gi, (k0, g) in enumerate(KGROUPS_F8):
        h = 0 if gi < SPLIT_GI else 1
        accs = acc_ab[h]
        if g == 1:  # 16-partition remainder tile (half B's stop)
            nc.scalar.dma_start(out=xt[:, NKT - 1:NKT, :],
                                in_=xt_d[:, NKT - 1:NKT, :])
            slab_t = consts.tile([NTAIL, MSHARD], fp8)
            nc.scalar.dma_start(out=slab_t, in_=a_tail)
            for i, (c0, c1) in enumerate(MCH):
                nc.tensor.matmul(
                    accs[i],
                    lhsT=xt[0:16, NKT - 1, :],
                    rhs=slab_t[:, c0:c1],
                    start=False,
                    stop=True,
                )
            continue
        if gi > 0:
            nc.scalar.dma_start(out=xt[:, k0:k0 + g, :],
                                in_=xt_d[:, k0:k0 + g, :])
        slab = slabs.tile([KT, KQ_F8, MSHARD], fp8, name=f"slab{gi}",
                          tag="slab")
        nc.sync.dma_start(out=slab[:, :g, :],
                          in_=a_t[gi * KT:(gi + 1) * KT, :g, :])
        for sub in range(0, g - 1, 2):
            kt = k0 + sub
            for i, (c0, c1) in enumerate(MCH):
                nc.tensor.matmul(
                    accs[i],
                    lhsT=xt[:, kt:kt + 2, :],
                    rhs=slab[:, sub:sub + 2, c0:c1],
                    start=(kt == 0 or kt == SPLIT_KT),
                    stop=(kt == STOP_A and h == 0),
                    perf_mode=DR,
                )
        if gi == SPLIT_GI - 1:
            half_mul(0)          # DVE: overlaps half B's matmuls
        if gi == SPLIT_GI + 1:
            half_reduce(0)       # PE: p0 is ready by now, no queue stall

    half_mul(1)
    half_reduce(1)

    # t = (s_a + s_b) * (-r) + F, then o = x * (-b) + t, one out DMA
    ssum = epil.tile([MT, NMT], f32, bufs=1)
    nc.vector.tensor_add(ssum, sa_s, s_ps[:, NMT:])
    t_full = epil.tile([MT, NMT], f32, bufs=1)
    nc.vector.tensor_scalar(
        out=t_full, in0=ssum, scalar1=-R_CONST, scalar2=F_CONST,
        op0=mybir.AluOpType.mult, op1=mybir.AluOpType.add,
    )
    o_full = epil.tile([MT, NMT * DIM], f32, bufs=1)
    for mt in range(NMT):
        nc.vector.tensor_scalar(
            out=o_full[:, mt * DIM:(mt + 1) * DIM],
            in0=xs[:, mt * DIM:(mt + 1) * DIM], scalar1=bcol,
            scalar2=t_full[:, mt:mt + 1], op0=mybir.AluOpType.mult,
            op1=mybir.AluOpType.add,
        )
    nc.scalar.dma_start(out=out_d, in_=o_full)


def _body_x_stat(ctx, tc, a_t, a_l, xt_d, xt8_d, xs_d, xst_d, id_d, out_d,
                 mmdt, mm_dtype):
    """k-outer loop; x k-tiles are the stationary operand, A^T slabs stream
    as the moving operand (large free dim -> full-rate fp32r / bf16).
    Produces Ax^T in PSUM (3 chunk accumulators, one bank each); epilogue
    transposes x^T*Ax^T back via the PE.

    DMA streams in KQ-k-tile groups (~1.3-2.6 MB per transfer) to amortize
    per-DMA overhead; the stationary x is preloaded in per-group chunks on
    the gpsimd queue so the first matmul does not wait for the whole x.

    split: A and x decomposed as hi+lo bf16 pairs; A@x ~= A_hi@x_hi +
    A_lo@x_hi + A_hi@x_lo.  a_t holds [A_hi^T | A_lo^T] side by side; xt
    holds [x_hi | x_lo] per k-tile so the two x terms ride in ONE 128-wide
    stationary: pass A computes both x_hi@A_hi (psum rows 0:64) and
    x_lo@A_hi (rows 64:128) in a single moving sweep of the A_hi slab
    half; pass B computes x_hi@A_lo.

    splitf8: like split but A_lo is a SEPARATE fp8e4m3 tensor pre-scaled
    by A_LO_SCALE, and pass B runs all-fp8 (x in fp8) -- 3 bytes/element
    of A traffic instead of 4; epilogue rescales the pass-B accumulator."""
    import concourse.bass  # noqa: F401
    from concourse import mybir

    nc = tc.nc
    f32 = mybir.dt.float32
    fp8 = mybir.dt.float8e4
    split = mm_dtype in ("split", "splitf8")
    f8 = mm_dtype == "splitf8"

    consts = ctx.enter_context(tc.tile_pool(name="consts", bufs=1))
    slabs = ctx.enter_context(tc.tile_pool(name="slabs", bufs=6))
    psums = ctx.enter_context(tc.tile_pool(name="psums", bufs=1, space="PSUM"))
    ptp = ctx.enter_context(tc.tile_pool(name="ptp", bufs=2, space="PSUM"))
    epil = ctx.enter_context(tc.tile_pool(name="epil", bufs=2))

    # elements per k-row in the a_t tensor.  For splitf8 the hi (bf16) and
    # lo (fp8) halves are byte-packed into one bf16-typed stream:
    # per k-tile per partition = 1250 bf16 hi elems then 1250 fp8 lo bytes
    # (= 625 bf16-elem slots); pass B reads the lo region via bitcast.
    awid = 2 * MSHARD if (split and not f8) else MSHARD
    if f8:
        awid = MSHARD + MSHARD // 2  # 1875 bf16 elems per k-tile
    xwid = 2 * DIM if split else DIM  # stationary block width per k-tile

    xt = consts.tile([KT, NKT * xwid], mmdt)
    if f8:
        xt8 = consts.tile([KT, NKT * DIM], fp8)
    bcol = consts.tile([MT, 1], f32)
    nc.vector.memset(bcol, -B_CONST)

    accs = [psums.tile([xwid, c1 - c0], f32, name=f"acc{i}", tag=f"acc{i}")
            for i, (c0, c1) in enumerate(MCH)]
    if split:
        accs_lo = [psums.tile([DIM, c1 - c0], f32, name=f"accl{i}",
                              tag=f"accl{i}")
                   for i, (c0, c1) in enumerate(MCH)]

    for gi, (k0, g) in enumerate(KGROUPS):
        # stationary chunk for this group's k-tiles (gpsimd queue, overlaps
        # with the slab stream on the sync queue)
        nc.gpsimd.dma_start(
            out=xt[:, k0 * xwid:(k0 + g) * xwid],
            in_=xt_d[:, k0 * xwid:(k0 + g) * xwid],
        )
        if f8:
            nc.gpsimd.dma_start(
                out=xt8[:, k0 * DIM:(k0 + g) * DIM],
                in_=xt8_d[:, k0 * DIM:(k0 + g) * DIM],
            )
        slab = slabs.tile([KT, KQ * awid], mmdt, name=f"slab{gi}", tag="slab")
        nc.sync.dma_start(out=slab[:, :g * awid],
                          in_=a_t[gi * KT:(gi + 1) * KT, :g * awid])

        for sub in range(g):
            kt = k0 + sub
            xoff = kt * xwid
            base = sub * MSHARD if f8 else sub * awid
            for i, (c0, c1) in enumerate(MCH):
                # pass A: [x_hi | x_lo] (or plain x) against the A_hi half
                nc.tensor.matmul(
                    accs[i],
                    lhsT=xt[:, xoff:xoff + xwid],
                    rhs=slab[:, base + c0:base + c1],
                    start=(kt == 0),
                    stop=(kt == NKT - 1),
                )
            if split:
                for i, (c0, c1) in enumerate(MCH):
                    # pass B: x_hi (bf16) or x (fp8) against the A_lo half
                    if f8:
                        off = g * MSHARD + (sub * MSHARD + c0) // 2
                        rhs = slab[:, off:off + (c1 - c0) // 2].bitcast(fp8)
                        lo_lhs = xt8[:, kt * DIM:(kt + 1) * DIM]
                    else:
                        rhs = slab[:, base + MSHARD + c0:base + MSHARD + c1]
                        lo_lhs = xt[:, xoff:xoff + DIM]
                    nc.tensor.matmul(
                        accs_lo[i],
                        lhsT=lo_lhs,
                        rhs=rhs,
                        start=(kt == 0),
                        stop=(kt == NKT - 1),
                    )

    # epilogue-only constants: issued after the slab stream in program
    # order so they don't delay the first matmuls; they transfer during
    # the main loop and are ready long before the epilogue needs them.
    xs = consts.tile([MT, NMT * DIM], f32)
    nc.gpsimd.dma_start(out=xs, in_=xs_d)
    xst = consts.tile([DIM, MSHARD], f32)
    nc.gpsimd.dma_start(out=xst, in_=xst_d)
    ident = consts.tile([DIM, DIM], f32)
    nc.gpsimd.dma_start(out=ident, in_=id_d)

    # P = x^T * Ax^T  (elementwise), [64, 1250] in SBUF
    p_full = epil.tile([DIM, MSHARD], f32, bufs=1)
    for i, (c0, c1) in enumerate(MCH):
        w = c1 - c0
        if split:
            # only one PSUM operand allowed per DVE op -> chain via SBUF
            tsum = epil.tile([DIM, w], f32, name=f"tsum{i}", tag="tsum")
            nc.vector.tensor_copy(tsum, accs[i][0:DIM, :])
            nc.vector.tensor_add(tsum, tsum, accs[i][DIM:2 * DIM, :])
            if f8:
                tlo = epil.tile([DIM, w], f32, name=f"tlo{i}", tag="tlo")
                nc.vector.tensor_scalar(
                    out=tlo, in0=accs_lo[i], scalar1=1.0 / A_LO_SCALE,
                    scalar2=None, op0=mybir.AluOpType.mult)
                nc.vector.tensor_add(tsum, tsum, tlo)
            else:
                nc.vector.tensor_add(tsum, tsum, accs_lo[i])
            nc.vector.tensor_mul(p_full[:, c0:c1], xst[:, c0:c1], tsum)
        else:
            nc.vector.tensor_mul(p_full[:, c0:c1], xst[:, c0:c1], accs[i])

    for mt in range(NMT):
        pt = ptp.tile([MT, DIM], f32, name=f"pt{mt}", tag="pt")
        nc.tensor.transpose(
            out=pt, in_=p_full[:, mt * MT:(mt + 1) * MT], identity=ident,
        )
        s = epil.tile([MT, 1], f32, name=f"s{mt}", tag="s")
        nc.vector.tensor_reduce(
            out=s, in_=pt, axis=mybir.AxisListType.X, op=mybir.AluOpType.add,
        )
        t_col = epil.tile([MT, 1], f32, name=f"t{mt}", tag="t")
        # t = s * (-r) + F
        nc.vector.tensor_scalar(
            out=t_col, in0=s, scalar1=-R_CONST, scalar2=F_CONST,
            op0=mybir.AluOpType.mult, op1=mybir.AluOpType.add,
        )
        o = epil.tile([MT, DIM], f32, name=f"o{mt}", tag="o")
        nc.vector.tensor_scalar(
            out=o, in0=xs[:, mt * DIM:(mt + 1) * DIM], scalar1=bcol,
            scalar2=t_col, op0=mybir.AluOpType.mult, op1=mybir.AluOpType.add,
        )
        nc.sync.dma_start(out=out_d[mt * MT:(mt + 1) * MT, :], in_=o)


def build(layout=None, mm_dtype=None):
    layout = layout or DEFAULT_LAYOUT
    mm_dtype = mm_dtype or DEFAULT_MM_DTYPE
    key = (layout, mm_dtype)
    if key in _nc_cache:
        return _nc_cache[key]

    from contextlib import ExitStack
    import concourse.tile as tile
    from concourse import bacc

    mmdt, f32 = _dtypes(mm_dtype)

    nc = bacc.Bacc(
        "TRN2",
        target_bir_lowering=False,
        debug=False,
        enable_asserts=False,
        num_devices=NCORES,
        name=f"biochem_{layout}_{mm_dtype}",
    )
    from concourse import mybir

    if mm_dtype == "f8dr":
        a_t = nc.dram_tensor(
            "a_t", [(NG_F8 - 1) * KT, KQ_F8, MSHARD], mmdt,
            kind="ExternalInput").ap()
        a_tail = nc.dram_tensor(
            "a_tail", [NTAIL, MSHARD], mmdt, kind="ExternalInput").ap()
        xt_d = nc.dram_tensor(
            "xt", [KT, NKT, DIM], mmdt, kind="ExternalInput").ap()
        xs_d = nc.dram_tensor(
            "xs", [MT, NMT * DIM], f32, kind="ExternalInput").ap()
        xst_d = nc.dram_tensor(
            "xst", [DIM, MSHARD], f32, kind="ExternalInput").ap()
        # out in the host-tiled layout: row p, cols mt*64..: out[mt*125+p, :]
        out_d = nc.dram_tensor(
            "out", [MT, NMT * DIM], f32, kind="ExternalOutput").ap()

        with tile.TileContext(nc) as tc:
            with ExitStack() as ctx:
                _body_f8dr(ctx, tc, a_t, a_tail, xt_d, xs_d, xst_d, out_d)
        nc.compile()
        _nc_cache[key] = nc
        return nc

    split = mm_dtype in ("split", "splitf8")
    f8 = mm_dtype == "splitf8"
    awid = 2 * MSHARD if (split and not f8) else MSHARD
    if f8:
        awid = MSHARD + MSHARD // 2  # byte-packed hi(bf16)+lo(fp8)
    xwid = 2 * DIM if split else DIM
    # a_t is pre-tiled host-side into slab layout: row gi*128+p holds the
    # p-th partition of DMA group gi ([KQ consecutive k-rows] worth of data)
    a_t = nc.dram_tensor(
        "a_t", [NG * KT, KQ * awid], mmdt, kind="ExternalInput").ap()
    a_l = xt8_d = None
    if f8:
        xt8_d = nc.dram_tensor(
            "xt8", [KT, NKT * DIM], mybir.dt.float8e4,
            kind="ExternalInput").ap()
    xt_d = nc.dram_tensor("xt", [KT, NKT * xwid], mmdt, kind="ExternalInput").ap()
    xs_d = nc.dram_tensor("xs", [MT, NMT * DIM], f32, kind="ExternalInput").ap()
    if layout == "x_stat":
        xst_d = nc.dram_tensor("xst", [DIM, MSHARD], f32, kind="ExternalInput").ap()
        id_d = nc.dram_tensor("ident", [DIM, DIM], f32, kind="ExternalInput").ap()
    out_d = nc.dram_tensor("out", [MSHARD, DIM], f32, kind="ExternalOutput").ap()

    with tile.TileContext(nc) as tc:
        with ExitStack() as ctx:
            if layout == "x_stat":
                _body_x_stat(ctx, tc, a_t, a_l, xt_d, xt8_d, xs_d, xst_d,
                             id_d, out_d, mmdt, mm_dtype)
            else:
                raise ValueError(layout)
    nc.compile()
    _nc_cache[key] = nc
    return nc


def prepare_in_maps(x, A, layout=None, mm_dtype=None):
    layout = layout or DEFAULT_LAYOUT
    mm_dtype = mm_dtype or DEFAULT_MM_DTYPE
    np_mm = _np_mm_dtype(mm_dtype)

    x = np.asarray(x, np.float32)
    A = np.asarray(A, np.float32)

    if mm_dtype == "f8dr":
        xp = np.zeros((KPAD, DIM), np.float32)
        xp[:N] = x
        xt_np = np.ascontiguousarray(
            xp.reshape(NKT, KT, DIM).transpose(1, 0, 2)).astype(np_mm)
        in_maps = []
        for c in range(NCORES):
            sh = slice(c * MSHARD, (c + 1) * MSHARD)
            at8 = np.ascontiguousarray(A[sh].T).astype(np_mm)  # [N, MSHARD]
            a_t_c = np.zeros(((NG_F8 - 1) * KT, KQ_F8, MSHARD), np_mm)
            for gi, (k0, g) in enumerate(KGROUPS_F8[:-1]):
                blk = at8[k0 * KT:(k0 + g) * KT]
                a_t_c[gi * KT:(gi + 1) * KT, :g, :] = (
                    blk.reshape(g, KT, MSHARD).transpose(1, 0, 2))
            xs_c = np.ascontiguousarray(
                x[sh].reshape(NMT, MT, DIM).transpose(1, 0, 2)
                .reshape(MT, NMT * DIM))
            in_maps.append({
                "a_t": a_t_c,
                "a_tail": np.ascontiguousarray(at8[(NKT - 1) * KT:]),
                "xt": xt_np, "xs": xs_c,
                "xst": np.ascontiguousarray(x[sh].T),
            })
        return in_maps

    split = mm_dtype in ("split", "splitf8")
    f8 = mm_dtype == "splitf8"
    if f8:
        import ml_dtypes
        np_fp8 = np.dtype(ml_dtypes.float8_e4m3)

    def tile_k(arr):
        """[KPAD, W] -> [KT, NKT*W] SBUF layout, padded rows are zero."""
        w = arr.shape[1]
        xp = np.zeros((KPAD, w), arr.dtype)
        xp[:N] = arr
        return np.ascontiguousarray(
            xp.reshape(NKT, KT, w).transpose(1, 0, 2).reshape(KT, NKT * w)
        )

    xt8_np = None
    if split:
        x_hi = x.astype(np_mm)
        x_lo = (x - x_hi.astype(np.float32)).astype(np_mm)
        # per k-tile stationary block is [x_hi | x_lo], 128 wide
        xt_np = tile_k(np.concatenate([x_hi, x_lo], axis=1))
        if f8:
            xt8_np = tile_k(x.astype(np_fp8))
    else:
        xt_np = tile_k(x).astype(np_mm)

    ident = np.eye(DIM, dtype=np.float32)

    def tile_slabs(at):
        """[KPAD, W] -> [NG*128, KQ*W] host pre-tiling into slab layout:
        row gi*128+p, cols sub*W:(sub+1)*W  =  at[(k0+sub)*128 + p, :]
        for group gi=(k0, g); unused columns of small groups stay zero."""
        w = at.shape[1]
        out = np.zeros((NG * KT, KQ * w), at.dtype)
        for gi, (k0, g) in enumerate(KGROUPS):
            blk = at[k0 * KT:(k0 + g) * KT, :]
            out[gi * KT:(gi + 1) * KT, :g * w] = (
                blk.reshape(g, KT, w).transpose(1, 0, 2).reshape(KT, g * w)
            )
        return out

    def pad_k(at):
        out = np.zeros((KPAD, at.shape[1]), at.dtype)
        out[:N] = at
        return out

    in_maps = []
    for c in range(NCORES):
        sh = slice(c * MSHARD, (c + 1) * MSHARD)
        at_f32 = pad_k(np.ascontiguousarray(A[sh].T))
        if f8:
            a_hi = at_f32.astype(np_mm)
            a_lo = at_f32 - a_hi.astype(np.float32)
            hi_t = tile_slabs(a_hi)                                # bf16
            lo_t = tile_slabs((a_lo * A_LO_SCALE).astype(np_fp8))  # fp8
            # byte-pack: per group row block, [g*2500 B hi][g*1250 B lo]
            awid = MSHARD + MSHARD // 2
            a_t_c = np.zeros((NG * KT, KQ * awid), np_mm)
            ob = a_t_c.view(np.uint8)
            hb = hi_t.view(np.uint8)
            lb = lo_t.view(np.uint8)
            for gi, (k0, g) in enumerate(KGROUPS):
                r = slice(gi * KT, (gi + 1) * KT)
                ob[r, :g * 2 * MSHARD] = hb[r, :g * 2 * MSHARD]
                ob[r, g * 2 * MSHARD:g * 3 * MSHARD] = lb[r, :g * MSHARD]
        elif split:
            a_hi = at_f32.astype(np_mm)
            a_lo = (at_f32 - a_hi.astype(np.float32)).astype(np_mm)
            a_t_c = tile_slabs(np.concatenate([a_hi, a_lo], axis=1))
        else:
            a_t_c = tile_slabs(at_f32.astype(np_mm))
        xs_c = np.ascontiguousarray(
            x[sh].reshape(NMT, MT, DIM).transpose(1, 0, 2).reshape(MT, NMT * DIM)
        )
        m = {"a_t": a_t_c, "xt": xt_np, "xs": xs_c}
        if f8:
            m["xt8"] = xt8_np
        if layout == "x_stat":
            m["xst"] = np.ascontiguousarray(x[sh].T)
            m["ident"] = ident
        in_maps.append(m)
    return in_maps


def run(inputs, trace=False, layout=None, mm_dtype=None, **spmd_kwargs):
    """Returns (full_output [10000, 64] f32, BassKernelResults)."""
    from concourse.bass_utils import run_bass_kernel_spmd

    nc = build(layout, mm_dtype)
    in_maps = prepare_in_maps(inputs["x"], inputs["A"], layout, mm_dtype)
    res = run_bass_kernel_spmd(
        nc, in_maps, core_ids=list(range(NCORES)), trace=trace, **spmd_kwargs
    )
    mm_dtype = mm_dtype or DEFAULT_MM_DTYPE
    if mm_dtype == "f8dr":
        # out comes back tiled [125, 10*64]; un-tile to [1250, 64] per core
        out = np.concatenate([
            res.results[c]["out"].reshape(MT, NMT, DIM).transpose(1, 0, 2)
            .reshape(MSHARD, DIM)
            for c in range(NCORES)
        ], axis=0)
    else:
        out = np.concatenate(
            [res.results[c]["out"] for c in range(NCORES)], axis=0)
    return out, res


def kernel(t=None, x=None, A=None):
    out, _ = run({"x": x, "A": A})
    return out



# revision 23
# speedup vs baseline: 2.5246x; 1.0427x over previous
"""Trainium2 Bass kernel for nn_BiochemicalDiffusion.

Computes  out = F - B*x - r * rowsum(x * (A @ x))  for A:[10000,10000] f32,
x:[10000,64] f32, across 8 NeuronCores.

Sharding (all done host-side in this file):
  - A is sharded row-wise: core c gets rows [c*1250, (c+1)*1250).
  - The shard is passed pre-transposed (A_shard^T, [10000, 1250]) so the PE
    can contract over k directly: Ax_shard = A_shard^T.T @ x.
  - x is passed in full to every core (it is tiny), pre-tiled into the
    [128, 79*64] SBUF layout the matmul consumes.
  - Each core computes its [1250, 64] slice of the output; the host
    concatenates them.

Hardware note: PSUM accumulation groups must not share a PSUM bank — two
interleaved accumulation groups in one bank corrupt each other.  Both
layouts below keep one live accumulation group per bank.

Everything is hardcoded to the problem shapes; kernel.py is self-contained.
"""

import numpy as np

N = 10000
DIM = 64
NCORES = 8
MSHARD = N // NCORES  # 1250 rows of A / out per core
MT = 125              # m-tile (PSUM partition) size
NMT = MSHARD // MT    # 10 m-tiles per core
KT = 128              # k-tile (contraction) size
NKT = 79              # k-tiles covering the 10000 rows (last is 16+zeros)
KPAD = NKT * KT       # 10112 (rows 10000+ are zeros; they contribute 0)

F_CONST = 1.0
B_CONST = 0.1
R_CONST = 0.01

# m-chunks for the x-stationary layout (moving free dim >= 256 keeps fp32r
# at full rate; each chunk's accumulator owns one PSUM bank; widths must be
# EVEN -- fp32r matmul ISA restriction on innermost free counts)
MCH = [(0, 418), (418, 834), (834, 1250)]

# k-tile DMA groups: up to 4 k-tiles per transfer (~1.3-2.6 MB).  A^T is
# pre-tiled on the HOST into exactly this slab layout (group-major,
# partition-major inside a group) so each group is ONE flat contiguous
# 2D DMA -- large per-partition bursts, minimal descriptor work.  The
# first groups are deliberately small so the first matmul starts early
# (pipeline ramp), the steady state uses full quads.
KQ = 4
KGROUPS = ([(0, 1), (1, 1), (2, 2)]
           + [(k0, 4) for k0 in range(4, 76, 4)]
           + [(76, 3)])
NG = len(KGROUPS)                     # 22 groups covering 79 tiles

A_LO_SCALE = 512.0  # fp8 A_lo is stored pre-scaled into [-1, 1]

# f8dr variant: A entirely fp8e4 (1 B/elem of HBM traffic), matmuls in
# DoubleRow perf mode (2 k-tiles = 256 contraction rows per instruction at
# 0.5 cycles per output row).  k-tiles 0..77 ride in pairs; tile 78 (the
# 16-real-rows remainder) is a trailing single-row matmul.  Groups must be
# even-aligned and even-sized so pairs never straddle a slab boundary.
KQ_F8 = 8
KGROUPS_F8 = [(0, 2), (2, 2), (4, 4), (8, 6), (14, 8), (22, 8), (30, 8),
              (38, 8), (46, 8), (54, 8), (62, 8), (70, 8), (78, 1)]
NG_F8 = len(KGROUPS_F8)
SPLIT_GI = 8  # groups [0, SPLIT_GI) accumulate into half A, rest into half B
assert sum(g for _, g in KGROUPS_F8) == NKT
NTAIL = N - (NKT - 1) * KT  # 16 real k-rows in the last k-tile

DEFAULT_LAYOUT = "x_stat"    # only x_stat is implemented
DEFAULT_MM_DTYPE = "f8dr"    # "f32r" | "bf16" | "split" | "splitf8" | "f8dr"

_nc_cache = {}


def _dtypes(mm_dtype):
    from concourse import mybir
    mm = {
        "f32": mybir.dt.float32,
        "f32r": mybir.dt.float32r,
        "bf16": mybir.dt.bfloat16,
        "split": mybir.dt.bfloat16,
        "splitf8": mybir.dt.bfloat16,
        "f8dr": mybir.dt.float8e4,
    }[mm_dtype]
    return mm, mybir.dt.float32


def _np_mm_dtype(mm_dtype):
    if mm_dtype in ("bf16", "split", "splitf8"):
        import ml_dtypes
        return np.dtype(ml_dtypes.bfloat16)
    if mm_dtype == "f8dr":
        import ml_dtypes
        return np.dtype(ml_dtypes.float8_e4m3)
    return np.dtype(np.float32)


def _body_f8dr(ctx, tc, a_t, a_tail, xt_d, xs_d, xst_d, out_d):
    """All-fp8 A stream with DoubleRow matmuls and split-k epilogue overlap.

    Per slab group: one contiguous fp8 DMA ([128, g, 1250], up to 10 KB per
    partition line) on the sync HWDGE queue, then g/2 DoubleRow matmuls per
    m-chunk, each consuming 2 k-tiles (256 contraction rows) of both the
    stationary x and the moving A^T slab via 3D APs [128, 2, w].  The
    remainder k-tile 78 (rows 9984..10000) is a 16-partition single-row
    matmul from its own tiny dram tensor.

    k is split in two halves with separate PSUM accumulator sets (6 banks);
    half A's epilogue (p = x^T*Ax^T, transpose, rowsum) runs on DVE/PE while
    half B's slabs still stream, so only half B's epilogue is tail-serial.
    x in [125, 640] layout is derived on-chip from x^T via PE transposes
    instead of a separate DMA.  All non-A traffic rides the scalar HWDGE
    queue; gpsimd (software DGE, slow drains) is unused."""
    import concourse.bass  # noqa: F401
    from concourse import mybir

    nc = tc.nc
    f32 = mybir.dt.float32
    fp8 = mybir.dt.float8e4
    DR = mybir.MatmulPerfMode.DoubleRow

    consts = ctx.enter_context(tc.tile_pool(name="consts", bufs=1))
    slabs = ctx.enter_context(tc.tile_pool(name="slabs", bufs=8))
    psums = ctx.enter_context(tc.tile_pool(name="psums", bufs=1, space="PSUM"))
    epil = ctx.enter_context(tc.tile_pool(name="epil", bufs=2))

    # scalar-queue constants; xt first (first matmul needs it) and in ONE
    # transfer -- per-group chunk writes into one tile created false
    # write-after-read dependency ladders that stalled the whole stream
    xt = consts.tile([KT, NKT, DIM], fp8)
    nc.scalar.dma_start(out=xt, in_=xt_d)
    xst = consts.tile([DIM, MSHARD], f32)
    nc.scalar.dma_start(out=xst, in_=xst_d)
    xs = consts.tile([MT, NMT * DIM], f32)
    nc.scalar.dma_start(out=xs, in_=xs_d)

    bcol = consts.tile([MT, 1], f32)
    nc.vector.memset(bcol, -B_CONST)
    ones = consts.tile([DIM, 1], f32)
    nc.vector.memset(ones, 1.0)

    acc_ab = [
        [psums.tile([DIM, c1 - c0], f32, name=f"acc{h}{i}", tag=f"acc{h}{i}")
         for i, (c0, c1) in enumerate(MCH)]
        for h in range(2)
    ]
    # s = rowsum(x^T * Ax^T) columns land here via ones-matmuls; 20
    # single-shot groups, strictly sequential, one bank
    s_ps = psums.tile([MT, 2 * NMT], f32)
    SPLIT_KT = KGROUPS_F8[SPLIT_GI][0]
    STOP_A = KGROUPS_F8[SPLIT_GI][0] - 2  # first kt of the last A pair

    p_ab = [None, None]
    sa_s = consts.tile([MT, NMT], f32)

    def half_mul(h):
        # p = x^T * Ax^T elementwise (DVE, reads the stopped accumulators)
        p = epil.tile([DIM, MSHARD], f32, name=f"p{h}", tag=f"p{h}", bufs=1)
        p_ab[h] = p
        for i, (c0, c1) in enumerate(MCH):
            nc.vector.tensor_mul(p[:, c0:c1], xst[:, c0:c1], acc_ab[h][i])

    def half_reduce(h):
        # s[:, mt] = p[:, mt-block]^T @ ones  (PE rowsum, no transposes)
        p = p_ab[h]
        for mt in range(NMT):
            col = h * NMT + mt
            nc.tensor.matmul(
                s_ps[:, col:col + 1],
                lhsT=p[:, mt * MT:(mt + 1) * MT],
                rhs=ones,
                start=True,
                stop=True,
            )
        if h == 0:
            # park half A's sums in SBUF so the final add has only one
            # PSUM operand
            nc.vector.tensor_copy(sa_s, s_ps[:, :NMT])

    for gi, (k0, g) in enumerate(KGROUPS_F8):
        h = 0 if gi < SPLIT_GI else 1
        accs = acc_ab[h]
        if g == 1:  # 16-partition remainder tile (half B's stop)
            slab_t = consts.tile([NTAIL, MSHARD], fp8)
            nc.scalar.dma_start(out=slab_t, in_=a_tail)
            for i, (c0, c1) in enumerate(MCH):
                nc.tensor.matmul(
                    accs[i],
                    lhsT=xt[0:16, NKT - 1, :],
                    rhs=slab_t[:, c0:c1],
                    start=False,
                    stop=True,
                )
            continue
        slab = slabs.tile([KT, KQ_F8, MSHARD], fp8, name=f"slab{gi}",
                          tag="slab")
        nc.sync.dma_start(out=slab[:, :g, :],
                          in_=a_t[gi * KT:(gi + 1) * KT, :g, :])
        for sub in range(0, g - 1, 2):
            kt = k0 + sub
            for i, (c0, c1) in enumerate(MCH):
                nc.tensor.matmul(
                    accs[i],
                    lhsT=xt[:, kt:kt + 2, :],
                    rhs=slab[:, sub:sub + 2, c0:c1],
                    start=(kt == 0 or kt == SPLIT_KT),
                    stop=(kt == STOP_A and h == 0),
                    perf_mode=DR,
                )
        if gi == SPLIT_GI - 1:
            half_mul(0)          # DVE: overlaps half B's matmuls
        if gi == SPLIT_GI + 1:
            half_reduce(0)       # PE: p0 is ready by now, no queue stall

    half_mul(1)
    half_reduce(1)

    # t = (s_a + s_b) * (-r) + F, then o = x * (-b) + t, one out DMA
    ssum = epil.tile([MT, NMT], f32, bufs=1)
    nc.vector.tensor_add(ssum, sa_s, s_ps[:, NMT:])
    t_full = epil.tile([MT, NMT], f32, bufs=1)
    nc.vector.tensor_scalar(
        out=t_full, in0=ssum, scalar1=-R_CONST, scalar2=F_CONST,
        op0=mybir.AluOpType.mult, op1=mybir.AluOpType.add,
    )
    o_full = epil.tile([MT, NMT * DIM], f32, bufs=1)
    for mt in range(NMT):
        # o = x * (-b) + t; odd m-tiles on the activation engine, even on
        # DVE, halving the serialized chain
        if mt % 2 == 0:
            nc.vector.tensor_scalar(
                out=o_full[:, mt * DIM:(mt + 1) * DIM],
                in0=xs[:, mt * DIM:(mt + 1) * DIM], scalar1=bcol,
                scalar2=t_full[:, mt:mt + 1], op0=mybir.AluOpType.mult,
                op1=mybir.AluOpType.add,
            )
        else:
            nc.scalar.activation(
                out=o_full[:, mt * DIM:(mt + 1) * DIM],
                in_=xs[:, mt * DIM:(mt + 1) * DIM],
                func=mybir.ActivationFunctionType.Identity,
                scale=-B_CONST, bias=t_full[:, mt:mt + 1],
            )
    nc.sync.dma_start(out=out_d, in_=o_full)


def _body_x_stat(ctx, tc, a_t, a_l, xt_d, xt8_d, xs_d, xst_d, id_d, out_d,
                 mmdt, mm_dtype):
    """k-outer loop; x k-tiles are the stationary operand, A^T slabs stream
    as the moving operand (large free dim -> full-rate fp32r / bf16).
    Produces Ax^T in PSUM (3 chunk accumulators, one bank each); epilogue
    transposes x^T*Ax^T back via the PE.

    DMA streams in KQ-k-tile groups (~1.3-2.6 MB per transfer) to amortize
    per-DMA overhead; the stationary x is preloaded in per-group chunks on
    the gpsimd queue so the first matmul does not wait for the whole x.

    split: A and x decomposed as hi+lo bf16 pairs; A@x ~= A_hi@x_hi +
    A_lo@x_hi + A_hi@x_lo.  a_t holds [A_hi^T | A_lo^T] side by side; xt
    holds [x_hi | x_lo] per k-tile so the two x terms ride in ONE 128-wide
    stationary: pass A computes both x_hi@A_hi (psum rows 0:64) and
    x_lo@A_hi (rows 64:128) in a single moving sweep of the A_hi slab
    half; pass B computes x_hi@A_lo.

    splitf8: like split but A_lo is a SEPARATE fp8e4m3 tensor pre-scaled
    by A_LO_SCALE, and pass B runs all-fp8 (x in fp8) -- 3 bytes/element
    of A traffic instead of 4; epilogue rescales the pass-B accumulator."""
    import concourse.bass  # noqa: F401
    from concourse import mybir

    nc = tc.nc
    f32 = mybir.dt.float32
    fp8 = mybir.dt.float8e4
    split = mm_dtype in ("split", "splitf8")
    f8 = mm_dtype == "splitf8"

    consts = ctx.enter_context(tc.tile_pool(name="consts", bufs=1))
    slabs = ctx.enter_context(tc.tile_pool(name="slabs", bufs=6))
    psums = ctx.enter_context(tc.tile_pool(name="psums", bufs=1, space="PSUM"))
    ptp = ctx.enter_context(tc.tile_pool(name="ptp", bufs=2, space="PSUM"))
    epil = ctx.enter_context(tc.tile_pool(name="epil", bufs=2))

    # elements per k-row in the a_t tensor.  For splitf8 the hi (bf16) and
    # lo (fp8) halves are byte-packed into one bf16-typed stream:
    # per k-tile per partition = 1250 bf16 hi elems then 1250 fp8 lo bytes
    # (= 625 bf16-elem slots); pass B reads the lo region via bitcast.
    awid = 2 * MSHARD if (split and not f8) else MSHARD
    if f8:
        awid = MSHARD + MSHARD // 2  # 1875 bf16 elems per k-tile
    xwid = 2 * DIM if split else DIM  # stationary block width per k-tile

    xt = consts.tile([KT, NKT * xwid], mmdt)
    if f8:
        xt8 = consts.tile([KT, NKT * DIM], fp8)
    bcol = consts.tile([MT, 1], f32)
    nc.vector.memset(bcol, -B_CONST)

    accs = [psums.tile([xwid, c1 - c0], f32, name=f"acc{i}", tag=f"acc{i}")
            for i, (c0, c1) in enumerate(MCH)]
    if split:
        accs_lo = [psums.tile([DIM, c1 - c0], f32, name=f"accl{i}",
                              tag=f"accl{i}")
                   for i, (c0, c1) in enumerate(MCH)]

    for gi, (k0, g) in enumerate(KGROUPS):
        # stationary chunk for this group's k-tiles (gpsimd queue, overlaps
        # with the slab stream on the sync queue)
        nc.gpsimd.dma_start(
            out=xt[:, k0 * xwid:(k0 + g) * xwid],
            in_=xt_d[:, k0 * xwid:(k0 + g) * xwid],
        )
        if f8:
            nc.gpsimd.dma_start(
                out=xt8[:, k0 * DIM:(k0 + g) * DIM],
                in_=xt8_d[:, k0 * DIM:(k0 + g) * DIM],
            )
        slab = slabs.tile([KT, KQ * awid], mmdt, name=f"slab{gi}", tag="slab")
        nc.sync.dma_start(out=slab[:, :g * awid],
                          in_=a_t[gi * KT:(gi + 1) * KT, :g * awid])

        for sub in range(g):
            kt = k0 + sub
            xoff = kt * xwid
            base = sub * MSHARD if f8 else sub * awid
            for i, (c0, c1) in enumerate(MCH):
                # pass A: [x_hi | x_lo] (or plain x) against the A_hi half
                nc.tensor.matmul(
                    accs[i],
                    lhsT=xt[:, xoff:xoff + xwid],
                    rhs=slab[:, base + c0:base + c1],
                    start=(kt == 0),
                    stop=(kt == NKT - 1),
                )
            if split:
                for i, (c0, c1) in enumerate(MCH):
                    # pass B: x_hi (bf16) or x (fp8) against the A_lo half
                    if f8:
                        off = g * MSHARD + (sub * MSHARD + c0) // 2
                        rhs = slab[:, off:off + (c1 - c0) // 2].bitcast(fp8)
                        lo_lhs = xt8[:, kt * DIM:(kt + 1) * DIM]
                    else:
                        rhs = slab[:, base + MSHARD + c0:base + MSHARD + c1]
                        lo_lhs = xt[:, xoff:xoff + DIM]
                    nc.tensor.matmul(
                        accs_lo[i],
                        lhsT=lo_lhs,
                        rhs=rhs,
                        start=(kt == 0),
                        stop=(kt == NKT - 1),
                    )

    # epilogue-only constants: issued after the slab stream in program
    # order so they don't delay the first matmuls; they transfer during
    # the main loop and are ready long before the epilogue needs them.
    xs = consts.tile([MT, NMT * DIM], f32)
    nc.gpsimd.dma_start(out=xs, in_=xs_d)
    xst = consts.tile([DIM, MSHARD], f32)
    nc.gpsimd.dma_start(out=xst, in_=xst_d)
    ident = consts.tile([DIM, DIM], f32)
    nc.gpsimd.dma_start(out=ident, in_=id_d)

    # P = x^T * Ax^T  (elementwise), [64, 1250] in SBUF
    p_full = epil.tile([DIM, MSHARD], f32, bufs=1)
    for i, (c0, c1) in enumerate(MCH):
        w = c1 - c0
        if split:
            # only one PSUM operand allowed per DVE op -> chain via SBUF
            tsum = epil.tile([DIM, w], f32, name=f"tsum{i}", tag="tsum")
            nc.vector.tensor_copy(tsum, accs[i][0:DIM, :])
            nc.vector.tensor_add(tsum, tsum, accs[i][DIM:2 * DIM, :])
            if f8:
                tlo = epil.tile([DIM, w], f32, name=f"tlo{i}", tag="tlo")
                nc.vector.tensor_scalar(
                    out=tlo, in0=accs_lo[i], scalar1=1.0 / A_LO_SCALE,
                    scalar2=None, op0=mybir.AluOpType.mult)
                nc.vector.tensor_add(tsum, tsum, tlo)
            else:
                nc.vector.tensor_add(tsum, tsum, accs_lo[i])
            nc.vector.tensor_mul(p_full[:, c0:c1], xst[:, c0:c1], tsum)
        else:
            nc.vector.tensor_mul(p_full[:, c0:c1], xst[:, c0:c1], accs[i])

    for mt in range(NMT):
        pt = ptp.tile([MT, DIM], f32, name=f"pt{mt}", tag="pt")
        nc.tensor.transpose(
            out=pt, in_=p_full[:, mt * MT:(mt + 1) * MT], identity=ident,
        )
        s = epil.tile([MT, 1], f32, name=f"s{mt}", tag="s")
        nc.vector.tensor_reduce(
            out=s, in_=pt, axis=mybir.AxisListType.X, op=mybir.AluOpType.add,
        )
        t_col = epil.tile([MT, 1], f32, name=f"t{mt}", tag="t")
        # t = s * (-r) + F
        nc.vector.tensor_scalar(
            out=t_col, in0=s, scalar1=-R_CONST, scalar2=F_CONST,
            op0=mybir.AluOpType.mult, op1=mybir.AluOpType.add,
        )
        o = epil.tile([MT, DIM], f32, name=f"o{mt}", tag="o")
        nc.vector.tensor_scalar(
            out=o, in0=xs[:, mt * DIM:(mt + 1) * DIM], scalar1=bcol,
            scalar2=t_col, op0=mybir.AluOpType.mult, op1=mybir.AluOpType.add,
        )
        nc.sync.dma_start(out=out_d[mt * MT:(mt + 1) * MT, :], in_=o)


def build(layout=None, mm_dtype=None):
    layout = layout or DEFAULT_LAYOUT
    mm_dtype = mm_dtype or DEFAULT_MM_DTYPE
    key = (layout, mm_dtype)
    if key in _nc_cache:
        return _nc_cache[key]

    from contextlib import ExitStack
    import concourse.tile as tile
    from concourse import bacc

    mmdt, f32 = _dtypes(mm_dtype)

    nc = bacc.Bacc(
        "TRN2",
        target_bir_lowering=False,
        debug=False,
        enable_asserts=False,
        num_devices=NCORES,
        name=f"biochem_{layout}_{mm_dtype}",
    )
    from concourse import mybir

    if mm_dtype == "f8dr":
        a_t = nc.dram_tensor(
            "a_t", [(NG_F8 - 1) * KT, KQ_F8, MSHARD], mmdt,
            kind="ExternalInput").ap()
        a_tail = nc.dram_tensor(
            "a_tail", [NTAIL, MSHARD], mmdt, kind="ExternalInput").ap()
        xt_d = nc.dram_tensor(
            "xt", [KT, NKT, DIM], mmdt, kind="ExternalInput").ap()
        xs_d = nc.dram_tensor(
            "xs", [MT, NMT * DIM], f32, kind="ExternalInput").ap()
        xst_d = nc.dram_tensor(
            "xst", [DIM, MSHARD], f32, kind="ExternalInput").ap()
        # out in the host-tiled layout: row p, cols mt*64..: out[mt*125+p, :]
        out_d = nc.dram_tensor(
            "out", [MT, NMT * DIM], f32, kind="ExternalOutput").ap()

        with tile.TileContext(nc) as tc:
            with ExitStack() as ctx:
                _body_f8dr(ctx, tc, a_t, a_tail, xt_d, xs_d, xst_d, out_d)
        nc.compile()
        _nc_cache[key] = nc
        return nc

    split = mm_dtype in ("split", "splitf8")
    f8 = mm_dtype == "splitf8"
    awid = 2 * MSHARD if (split and not f8) else MSHARD
    if f8:
        awid = MSHARD + MSHARD // 2  # byte-packed hi(bf16)+lo(fp8)
    xwid = 2 * DIM if split else DIM
    # a_t is pre-tiled host-side into slab layout: row gi*128+p holds the
    # p-th partition of DMA group gi ([KQ consecutive k-rows] worth of data)
    a_t = nc.dram_tensor(
        "a_t", [NG * KT, KQ * awid], mmdt, kind="ExternalInput").ap()
    a_l = xt8_d = None
    if f8:
        xt8_d = nc.dram_tensor(
            "xt8", [KT, NKT * DIM], mybir.dt.float8e4,
            kind="ExternalInput").ap()
    xt_d = nc.dram_tensor("xt", [KT, NKT * xwid], mmdt, kind="ExternalInput").ap()
    xs_d = nc.dram_tensor("xs", [MT, NMT * DIM], f32, kind="ExternalInput").ap()
    if layout == "x_stat":
        xst_d = nc.dram_tensor("xst", [DIM, MSHARD], f32, kind="ExternalInput").ap()
        id_d = nc.dram_tensor("ident", [DIM, DIM], f32, kind="ExternalInput").ap()
    out_d = nc.dram_tensor("out", [MSHARD, DIM], f32, kind="ExternalOutput").ap()

    with tile.TileContext(nc) as tc:
        with ExitStack() as ctx:
            if layout == "x_stat":
                _body_x_stat(ctx, tc, a_t, a_l, xt_d, xt8_d, xs_d, xst_d,
                             id_d, out_d, mmdt, mm_dtype)
            else:
                raise ValueError(layout)
    nc.compile()
    _nc_cache[key] = nc
    return nc


def prepare_in_maps(x, A, layout=None, mm_dtype=None):
    layout = layout or DEFAULT_LAYOUT
    mm_dtype = mm_dtype or DEFAULT_MM_DTYPE
    np_mm = _np_mm_dtype(mm_dtype)

    x = np.asarray(x, np.float32)
    A = np.asarray(A, np.float32)

    if mm_dtype == "f8dr":
        xp = np.zeros((KPAD, DIM), np.float32)
        xp[:N] = x
        xt_np = np.ascontiguousarray(
            xp.reshape(NKT, KT, DIM).transpose(1, 0, 2)).astype(np_mm)
        in_maps = []
        for c in range(NCORES):
            sh = slice(c * MSHARD, (c + 1) * MSHARD)
            at8 = np.ascontiguousarray(A[sh].T).astype(np_mm)  # [N, MSHARD]
            a_t_c = np.zeros(((NG_F8 - 1) * KT, KQ_F8, MSHARD), np_mm)
            for gi, (k0, g) in enumerate(KGROUPS_F8[:-1]):
                blk = at8[k0 * KT:(k0 + g) * KT]
                a_t_c[gi * KT:(gi + 1) * KT, :g, :] = (
                    blk.reshape(g, KT, MSHARD).transpose(1, 0, 2))
            xs_c = np.ascontiguousarray(
                x[sh].reshape(NMT, MT, DIM).transpose(1, 0, 2)
                .reshape(MT, NMT * DIM))
            in_maps.append({
                "a_t": a_t_c,
                "a_tail": np.ascontiguousarray(at8[(NKT - 1) * KT:]),
                "xt": xt_np, "xs": xs_c,
                "xst": np.ascontiguousarray(x[sh].T),
            })
        return in_maps

    split = mm_dtype in ("split", "splitf8")
    f8 = mm_dtype == "splitf8"
    if f8:
        import ml_dtypes
        np_fp8 = np.dtype(ml_dtypes.float8_e4m3)

    def tile_k(arr):
        """[KPAD, W] -> [KT, NKT*W] SBUF layout, padded rows are zero."""
        w = arr.shape[1]
        xp = np.zeros((KPAD, w), arr.dtype)
        xp[:N] = arr
        return np.ascontiguousarray(
            xp.reshape(NKT, KT, w).transpose(1, 0, 2).reshape(KT, NKT * w)
        )

    xt8_np = None
    if split:
        x_hi = x.astype(np_mm)
        x_lo = (x - x_hi.astype(np.float32)).astype(np_mm)
        # per k-tile stationary block is [x_hi | x_lo], 128 wide
        xt_np = tile_k(np.concatenate([x_hi, x_lo], axis=1))
        if f8:
            xt8_np = tile_k(x.astype(np_fp8))
    else:
        xt_np = tile_k(x).astype(np_mm)

    ident = np.eye(DIM, dtype=np.float32)

    def tile_slabs(at):
        """[KPAD, W] -> [NG*128, KQ*W] host pre-tiling into slab layout:
        row gi*128+p, cols sub*W:(sub+1)*W  =  at[(k0+sub)*128 + p, :]
        for group gi=(k0, g); unused columns of small groups stay zero."""
        w = at.shape[1]
        out = np.zeros((NG * KT, KQ * w), at.dtype)
        for gi, (k0, g) in enumerate(KGROUPS):
            blk = at[k0 * KT:(k0 + g) * KT, :]
            out[gi * KT:(gi + 1) * KT, :g * w] = (
                blk.reshape(g, KT, w).transpose(1, 0, 2).reshape(KT, g * w)
            )
        return out

    def pad_k(at):
        out = np.zeros((KPAD, at.shape[1]), at.dtype)
        out[:N] = at
        return out

    in_maps = []
    for c in range(NCORES):
        sh = slice(c * MSHARD, (c + 1) * MSHARD)
        at_f32 = pad_k(np.ascontiguousarray(A[sh].T))
        if f8:
            a_hi = at_f32.astype(np_mm)
            a_lo = at_f32 - a_hi.astype(np.float32)
            hi_t = tile_slabs(a_hi)                                # bf16
            lo_t = tile_slabs((a_lo * A_LO_SCALE).astype(np_fp8))  # fp8
            # byte-pack: per group row block, [g*2500 B hi][g*1250 B lo]
            awid = MSHARD + MSHARD // 2
            a_t_c = np.zeros((NG * KT, KQ * awid), np_mm)
            ob = a_t_c.view(np.uint8)
            hb = hi_t.view(np.uint8)
            lb = lo_t.view(np.uint8)
            for gi, (k0, g) in enumerate(KGROUPS):
                r = slice(gi * KT, (gi + 1) * KT)
                ob[r, :g * 2 * MSHARD] = hb[r, :g * 2 * MSHARD]
                ob[r, g * 2 * MSHARD:g * 3 * MSHARD] = lb[r, :g * MSHARD]
        elif split:
            a_hi = at_f32.astype(np_mm)
            a_lo = (at_f32 - a_hi.astype(np.float32)).astype(np_mm)
            a_t_c = tile_slabs(np.concatenate([a_hi, a_lo], axis=1))
        else:
            a_t_c = tile_slabs(at_f32.astype(np_mm))
        xs_c = np.ascontiguousarray(
            x[sh].reshape(NMT, MT, DIM).transpose(1, 0, 2).reshape(MT, NMT * DIM)
        )
        m = {"a_t": a_t_c, "xt": xt_np, "xs": xs_c}
        if f8:
            m["xt8"] = xt8_np
        if layout == "x_stat":
            m["xst"] = np.ascontiguousarray(x[sh].T)
            m["ident"] = ident
        in_maps.append(m)
    return in_maps


def run(inputs, trace=False, layout=None, mm_dtype=None, **spmd_kwargs):
    """Returns (full_output [10000, 64] f32, BassKernelResults)."""
    from concourse.bass_utils import run_bass_kernel_spmd

    nc = build(layout, mm_dtype)
    in_maps = prepare_in_maps(inputs["x"], inputs["A"], layout, mm_dtype)
    res = run_bass_kernel_spmd(
        nc, in_maps, core_ids=list(range(NCORES)), trace=trace, **spmd_kwargs
    )
    mm_dtype = mm_dtype or DEFAULT_MM_DTYPE
    if mm_dtype == "f8dr":
        # out comes back tiled [125, 10*64]; un-tile to [1250, 64] per core
        out = np.concatenate([
            res.results[c]["out"].reshape(MT, NMT, DIM).transpose(1, 0, 2)
            .reshape(MSHARD, DIM)
            for c in range(NCORES)
        ], axis=0)
    else:
        out = np.concatenate(
            [res.results[c]["out"] for c in range(NCORES)], axis=0)
    return out, res


def kernel(t=None, x=None, A=None):
    out, _ = run({"x": x, "A": A})
    return out

